# revision 1
# baseline (speedup 1.0000x reference)
"""Causal self-attention Trainium2 kernel.

Problem: B=4, T=2048, D=1024, H=16 heads (hd=64).
Sharding: 8 cores; core c -> batch c//2, heads (c%2)*8 .. +8.
Each core computes a partial output projection (its 512 rows of w_proj);
host sums the two partials per batch and adds b_proj.

Layout strategy (per core):
  - x^T [D, T] streamed in fp32, consumed as float32r (full-rate matmuls
    at near-fp32 precision for the QKV projections; host pre-transposed).
  - Q^T, K^T computed as [512, 2048] (head-dim on partitions) via
    W-stationary matmuls: out = W_chunk.T @ x^T, stored bf16.
  - V computed in natural [T, 512] layout (x^T-stationary), stored per-head
    augmented with a ones column -> [128k, head, 65], so the P@V matmul
    accumulates softmax denominators for free in row 64.
  - Scores computed transposed: S^T[k, q] = (K^T_chunk).T @ Q^T, causal
    blocks only; exp on ScalarE straight out of PSUM (no max subtraction --
    scaled scores are ~N(0,1), max << 88); triangular mask multiply only on
    diagonal 128-blocks.
  - P@V with V_aug stationary: out^T[65, q] accumulated over k-chunks in
    PSUM. Row 64 = sum of exp. Normalize with DVE reciprocal + GpSimd
    partition_broadcast; result written as A^T [512, 2048] bf16 which is
    exactly the lhsT needed for the output projection.

Schedule strategy (this revision):
  - All input DMAs are batched into few multi-level-AP copies and split
    across the SP and Activation HWDGE queues (plus Pool SWDGE for small
    constants) so dispatch overhead (~650ns/copy/queue) never gates the
    startup.
  - All projection work (V, K/Q chunks, output projection) is expressed as
    generators yielding one matmul at a time.  A deadline-ordered filler
    queue is pumped a few matmuls per attention k-step, so the Tensor
    engine always has ready work while the Exp activation (which otherwise
    paces the attention inner loop at ~1.04us per 128-k-block) runs ahead.
  - The P@V matmul for block ki is emitted one iteration late (software
    pipelining) so it never blocks the in-order PE queue waiting on exp.
"""

import sys
from collections import deque

for _p in ("/opt/trn_rl_repo",):
    if _p not in sys.path:
        sys.path.insert(0, _p)

import numpy as np
import ml_dtypes

import concourse.bass as bass
import concourse.mybir as mybir
import concourse.tile as tile
from concourse import bacc
from concourse.bass_utils import run_bass_kernel_spmd

BF16 = ml_dtypes.bfloat16

B, T, D = 4, 2048, 1024
H, HD = 16, 64
NCORES = 8
HPC = 8                  # heads per core
GCOLS = HPC * HD         # 512 columns of qkv per core per q/k/v
P = 128
NDC = D // P             # 8 contraction chunks of 128
NTT = T // P             # 16 t-tiles of 128
NQC = T // 512           # 4 q-chunks of 512
NMC = GCOLS // P         # 4 M-chunks per Q^T / K^T


def _ap3(dram, off, part_stride, nparts, mid_stride, nmid, inner):
    """3-level DRAM access pattern: [partition, mid, contiguous-inner]."""
    a = dram.ap()
    return bass.AP(tensor=a.tensor, offset=a.offset + off,
                   ap=[[part_stride, nparts], [mid_stride, nmid], [1, inner]])


def build_nc(trace_sim: bool = False):
    f32 = mybir.dt.float32
    f32r = mybir.dt.float32r
    bf16 = mybir.dt.bfloat16

    nc = bacc.Bacc("TRN2", target_bir_lowering=False, debug=False,
                   num_devices=NCORES)

    xT_d = nc.dram_tensor("xT", [D, T], bf16, kind="ExternalInput")
    # wkq: interleaved 128-col blocks [K0 Q0 K1 Q1 K2 Q2 K3 Q3]
    wkq_d = nc.dram_tensor("wkq", [D, 2 * GCOLS], bf16, kind="ExternalInput")
    wv_d = nc.dram_tensor("wv", [D, GCOLS], bf16, kind="ExternalInput")
    wp_d = nc.dram_tensor("wp", [GCOLS, D], bf16, kind="ExternalInput")
    # bqk: col 2m = bias for K_m block, col 2m+1 = bias for Q_m block
    bqk_d = nc.dram_tensor("bqk", [P, 2 * NMC], f32, kind="ExternalInput")
    bv_d = nc.dram_tensor("bv", [GCOLS], f32, kind="ExternalInput")
    tri_d = nc.dram_tensor("tri", [P, P], bf16, kind="ExternalInput")
    out_d = nc.dram_tensor("outp", [T, D], bf16, kind="ExternalOutput")

    with tile.TileContext(nc, trace_sim=trace_sim) as tc:
        with (
            tc.tile_pool(name="consts", bufs=1) as consts,
            tc.tile_pool(name="weights", bufs=1) as weights,
            tc.tile_pool(name="acts", bufs=1) as acts,
            tc.tile_pool(name="pt", bufs=6) as ptp,
            tc.tile_pool(name="norm", bufs=4) as normp,
            tc.tile_pool(name="outs", bufs=3) as outsp,
            tc.tile_pool(name="ps_mm", bufs=2, space="PSUM") as ps_mm,
            tc.tile_pool(name="ps_st", bufs=2, space="PSUM") as ps_st,
            tc.tile_pool(name="ps_o", bufs=2, space="PSUM") as ps_o,
        ):
            wv_sb = weights.tile([P, NDC, GCOLS], bf16)
            xT_sb = acts.tile([P, NDC, T], bf16)
            wkq_sb = weights.tile([P, NDC, 2 * GCOLS], bf16)
            wp_sb = weights.tile([P, NMC, D], bf16)

            # ---- Single SP HWDGE stream in priority order: the HWDGE FIFO
            # is shared, so one in-order queue gives exact control of what
            # bytes land first.  V work (wv + x t-slices) unlocks the most
            # early PE work per byte; wkq m0 gates the first attention
            # block; bulk x and later wkq/wp chunks follow.
            def dma_x(c0, c1):
                nc.sync.dma_start(
                    xT_sb[:, :, c0:c1],
                    _ap3(xT_d, c0, T, P, P * T, NDC, c1 - c0))

            def dma_wkq(m):
                nc.sync.dma_start(
                    wkq_sb[:, :, 256 * m:256 * (m + 1)],
                    _ap3(wkq_d, 256 * m, 2 * GCOLS, P,
                         P * 2 * GCOLS, NDC, 256))

            def dma_x_dc(d0, d1, c0, c1):
                nc.sync.dma_start(
                    xT_sb[:, d0:d1, c0:c1],
                    _ap3(xT_d, d0 * P * T + c0, T, P, P * T, d1 - d0,
                         c1 - c0))

            def dma_wv(d0, d1):
                nc.sync.dma_start(
                    wv_sb[:, d0:d1, :],
                    _ap3(wv_d, d0 * P * GCOLS, GCOLS, P, P * GCOLS,
                         d1 - d0, GCOLS))

            # bf16: t-slices must be >=256 cols to keep 512B-contiguous
            # descriptors (full DMA rate)
            dma_x_dc(0, 4, 0, 256)
            dma_wv(0, 2)
            dma_x_dc(4, 8, 0, 256)
            dma_wv(2, 4)
            dma_x(256, 512)
            dma_wv(4, 6)
            dma_wv(6, 8)
            dma_wkq(0)
            dma_x(512, 1024)
            dma_x(1024, 1536)
            dma_x(1536, 2048)
            for m in range(1, NMC):
                dma_wkq(m)
            nc.sync.dma_start(wp_sb[:, :, :],
                              _ap3(wp_d, 0, D, P, P * D, NMC, D))
            # ---- Pool SWDGE: small constants ----
            bqk_sb = consts.tile([P, 2 * NMC], f32)
            nc.gpsimd.dma_start(bqk_sb[:], bqk_d.ap())
            tri_sb = consts.tile([P, P], bf16)
            nc.gpsimd.dma_start(tri_sb[:], tri_d.ap())
            bv_rep = consts.tile([P, GCOLS], f32)
            bv_ap = bv_d.ap()
            nc.gpsimd.dma_start(
                bv_rep[:],
                bass.AP(tensor=bv_ap.tensor, offset=bv_ap.offset,
                        ap=[[0, P]] + list(bv_ap.ap)),
            )

            # warm the ScalarE Exp table during the startup DMA window
            warm = consts.tile([1, 1], f32)
            nc.vector.memset(warm[:], 0.0)
            nc.scalar.activation(warm[:], warm[:],
                                 mybir.ActivationFunctionType.Exp)

            # V natural + ones column: [128, tt, head, 65]
            V_sb = acts.tile([P, NTT, HPC, HD + 1], bf16)
            nc.vector.memset(V_sb[:, :, :, HD], 1.0)

            QT_sb = acts.tile([P, NMC, T], bf16)
            KT_sb = acts.tile([P, NMC, T], bf16)
            AT_sb = acts.tile([P, NMC, T], bf16)
            # SBUF staging for unnormalized P@V blocks: the PSUM po slot is
            # released by a plain copy at the qc boundary; the normalize
            # (recip/broadcast/mult) is deferred into the next qc's ki loop
            # where the DVE/Pool queues are quiet
            stage_sb = acts.tile([P, 4, 2, 512], f32)

            # ---------------- filler generators ----------------
            def gen_v(tt):
                pv = ps_mm.tile([P, 512], f32, tag="mm", name=f"pv{tt}")
                for dc in range(NDC):
                    nc.tensor.matmul(
                        pv[:],
                        xT_sb[:, dc, tt * P:(tt + 1) * P],
                        wv_sb[:, dc, :],
                        start=(dc == 0), stop=(dc == NDC - 1),
                    )
                    if dc < NDC - 1:
                        yield
                # NOTE: PSUM readers must be PE/DVE/Act (GPSIMD cannot
                # access PSUM on hardware)
                nc.vector.tensor_tensor(
                    V_sb[:, tt, :, 0:HD],
                    pv[:].rearrange("p (h d) -> p h d", h=HPC),
                    bv_rep[:].rearrange("p (h d) -> p h d", h=HPC),
                    mybir.AluOpType.add,
                )

            def gen_kq(is_q, m, tc4):
                col = 256 * m + (128 if is_q else 0)
                pq = ps_mm.tile([P, 512], f32, tag="mm",
                                name=f"p{'q' if is_q else 'k'}{m}_{tc4}")
                for dc in range(NDC):
                    nc.tensor.matmul(
                        pq[:],
                        wkq_sb[:, dc, col:col + P],
                        xT_sb[:, dc, tc4 * 512:(tc4 + 1) * 512],
                        start=(dc == 0), stop=(dc == NDC - 1),
                    )
                    if dc < NDC - 1:
                        yield
                dst = QT_sb if is_q else KT_sb
                bcol = 2 * m + (1 if is_q else 0)
                nc.vector.tensor_scalar_add(
                    dst[:, m, tc4 * 512:(tc4 + 1) * 512],
                    pq[:], bqk_sb[:, bcol:bcol + 1],
                )

            def gen_out(tt, ncol, pool_tag=None, tail=False):
                pool, tag = pool_tag or (ps_mm, "mm")
                pp = pool.tile([P, 512], f32, tag=tag,
                               name=f"pp{tt}_{ncol}")
                for hc in range(NMC):
                    nc.tensor.matmul(
                        pp[:],
                        AT_sb[:, hc, tt * P:(tt + 1) * P],
                        wp_sb[:, hc, ncol * 512:(ncol + 1) * 512],
                        start=(hc == 0), stop=(hc == NMC - 1),
                    )
                    if hc < NMC - 1:
                        yield
                ot = outsp.tile([P, 512], bf16, tag="ot", bufs=7)
                # PSUM->SBUF copy: DVE normally; groups whose copies land
                # near the end (when the DVE queue is jammed with the final
                # masks/normalize but the exp stream is winding down) use
                # Act's activation-Copy path instead
                if tail:
                    nc.scalar.activation(ot[:], pp[:],
                                         mybir.ActivationFunctionType.Copy)
                else:
                    nc.vector.tensor_copy(ot[:], pp[:])
                nc.sync.dma_start(
                    out_d[tt * P:(tt + 1) * P,
                          ncol * 512:(ncol + 1) * 512],
                    ot[:],
                )

            def gen_out_pair(tt, ptA, ptB, deng):
                """Tail variant: both 512-col halves of a tt row-block, one
                combined DMA (halves the tail HWDGE ladder)."""
                poolA, tagA = ptA
                poolB, tagB = ptB
                ppA = poolA.tile([P, 512], f32, tag=tagA, name=f"ppa{tt}")
                ppB = poolB.tile([P, 512], f32, tag=tagB, name=f"ppb{tt}")
                for hc in range(NMC):
                    for pp, ncol in ((ppA, 0), (ppB, 1)):
                        nc.tensor.matmul(
                            pp[:],
                            AT_sb[:, hc, tt * P:(tt + 1) * P],
                            wp_sb[:, hc, ncol * 512:(ncol + 1) * 512],
                            start=(hc == 0), stop=(hc == NMC - 1),
                        )
                        if not (hc == NMC - 1 and ncol == 1):
                            yield
                ot2 = outsp.tile([P, 1024], bf16, tag="ot2", bufs=4)
                nc.vector.tensor_copy(ot2[:, 0:512], ppA[:])
                nc.scalar.activation(ot2[:, 512:1024], ppB[:],
                                     mybir.ActivationFunctionType.Copy)
                deng.dma_start(out_d[tt * P:(tt + 1) * P, :], ot2[:])

            fillers = deque()   # (deadline, generator)

            def pump(n):
                while n > 0 and fillers:
                    try:
                        next(fillers[0][1])
                    except StopIteration:
                        fillers.popleft()
                    n -= 1

            def drain_until(deadline):
                while fillers and fillers[0][0] <= deadline:
                    for _ in fillers.popleft()[1]:
                        pass

            def drain_rr():
                # round-robin across remaining generators so independent
                # matmuls (early hc chunks of each out-proj group) are
                # emitted ahead of ones gated on the final normalize
                while fillers:
                    _, g = fillers.popleft()
                    try:
                        next(g)
                    except StopIteration:
                        continue
                    fillers.append((None, g))

            def run_gen(g):
                for _ in g:
                    pass

            # ---------------- startup compute ----------------
            for tt in range(4):
                run_gen(gen_v(tt))
            run_gen(gen_kq(False, 0, 0))
            run_gen(gen_kq(True, 0, 0))

            # deadline-ordered filler queue (deadline = (mch, qc) at whose
            # start the group's output is first consumed)
            for m in range(NMC):
                for qc in range(NQC):
                    if m == 0 and qc == 0:
                        continue
                    fillers.append(((m, qc), gen_kq(False, m, qc)))
                    fillers.append(((m, qc), gen_kq(True, m, qc)))
                    if m == 0:
                        for tt in range(4 * qc, 4 * qc + 4):
                            fillers.append(((m, qc), gen_v(tt)))

            pending_norm = []

            def do_norm():
                while pending_norm:
                    m_, q_ = pending_norm.pop(0)
                    st = stage_sb[:, q_ % 4]
                    for j in range(2):
                        part = j * 64
                        rs = normp.tile([1, 512], f32, tag="rs")
                        nc.vector.reciprocal(rs[:], st[HD:HD + 1, j, :])
                        rep = normp.tile([64, 512], f32, tag="rep")
                        nc.gpsimd.partition_broadcast(rep[:], rs[0:1, :])
                        nc.vector.tensor_tensor(
                            AT_sb[part:part + 64, m_,
                                  q_ * 512:(q_ + 1) * 512],
                            st[0:HD, j, :], rep[:], mybir.AluOpType.mult,
                        )

            # ---------------- attention main loop ----------------
            for mch in range(NMC):
                for qc in range(NQC):
                    drain_until((mch, qc))
                    po = [ps_o.tile([HD + 1, 512], f32, tag="po",
                                    name=f"po{mch}_{qc}_{j}")
                          for j in range(2)]
                    nki = 4 * qc + 4
                    prev = None
                    for ki in range(nki):
                        off = max(0, ki - 4 * qc) * P
                        pshat = ps_st.tile([P, 2, 512], f32, tag="st")
                        pts = ptp.tile([P, 2, 512], bf16, tag="pt")
                        for j in range(2):
                            part = j * 64
                            nc.tensor.matmul(
                                pshat[:, j, off:512],
                                KT_sb[part:part + 64, mch,
                                      ki * P:(ki + 1) * P],
                                QT_sb[part:part + 64, mch,
                                      qc * 512 + off:(qc + 1) * 512],
                                start=True, stop=True,
                            )
                        nc.scalar.activation(
                            pts[:, :, off:512], pshat[:, :, off:512],
                            mybir.ActivationFunctionType.Exp,
                            scale=0.125,
                        )
                        if ki >= 4 * qc:
                            # diagonal block: zero out q < k entries
                            for j in range(2):
                                nc.vector.tensor_tensor(
                                    pts[:, j, off:off + P],
                                    pts[:, j, off:off + P],
                                    tri_sb[:], mybir.AluOpType.mult,
                                )
                        if ki == 1:
                            do_norm()
                        pump(3 if mch == NMC - 1 else 2)
                        if prev is not None:
                            poff, ppts = prev
                            for j in range(2):
                                nc.tensor.matmul(
                                    po[j][:, poff:512],
                                    V_sb[:, ki - 1, 2 * mch + j, :],
                                    ppts[:, j, poff:512],
                                    start=(ki - 1 == 0), stop=False,
                                )
                        prev = (off, pts)
                    poff, ppts = prev
                    for j in range(2):
                        nc.tensor.matmul(
                            po[j][:, poff:512],
                            V_sb[:, nki - 1, 2 * mch + j, :],
                            ppts[:, j, poff:512],
                            start=(nki == 1), stop=True,
                        )
                    if mch == NMC - 1 and qc == NQC - 1:
                        # last block: normalize straight from PSUM -- the
                        # staging copy would only lengthen the tail's
                        # critical chain
                        for j in range(2):
                            part = j * 64
                            rs = normp.tile([1, 512], f32, tag="rs")
                            nc.vector.reciprocal(rs[:], po[j][HD:HD + 1, :])
                            rep = normp.tile([64, 512], f32, tag="rep")
                            nc.gpsimd.partition_broadcast(rep[:], rs[0:1, :])
                            nc.vector.tensor_tensor(
                                AT_sb[part:part + 64, mch,
                                      qc * 512:(qc + 1) * 512],
                                po[j][0:HD, :], rep[:],
                                mybir.AluOpType.mult,
                            )
                    else:
                        # stage the unnormalized block out of PSUM (fast
                        # slot release); defer the normalize into the next
                        # qc's ki loop
                        st = stage_sb[:, qc % 4]
                        for j in range(2):
                            nc.vector.tensor_copy(st[0:HD + 1, j, :],
                                                  po[j][0:HD + 1, :])
                        pending_norm.append((mch, qc))
                    if mch == NMC - 1:
                        if qc < NQC - 1:
                            for tt in range(4 * qc, 4 * qc + 4):
                                for ncol in range(2):
                                    fillers.append(((9, qc),
                                                    gen_out(tt, ncol)))
            # ---- tail: last qc's output projection in two waves ----
            # wave 1 uses st/mm slots (free as soon as the last exp /
            # pumped copies retire) so its hc0-2 matmuls fill the PE while
            # the final normalize chain runs; wave 2 (po slots, freed by
            # that normalize) follows
            st_, po_, mm_ = (ps_st, "st"), (ps_o, "po"), (ps_mm, "mm")
            t0 = 4 * (NQC - 1)
            wave1 = [g for _, g in fillers] + [
                gen_out_pair(t0 + 0, st_, st_, nc.sync),
                gen_out_pair(t0 + 1, mm_, mm_, nc.scalar),
            ]
            wave2 = [
                gen_out_pair(t0 + 2, po_, po_, nc.sync),
                gen_out_pair(t0 + 3, st_, mm_, nc.scalar),
            ]
            for wave in (wave1, wave2):
                wave = deque(wave)
                while wave:
                    g = wave.popleft()
                    try:
                        next(g)
                    except StopIteration:
                        continue
                    wave.append(g)

    nc.compile()
    return nc


def host_inputs(x, w_qkv, b_qkv):
    """Per-core input maps. Core c -> batch c//2, head group c%2."""
    x = np.asarray(x, np.float32)
    w_qkv = np.asarray(w_qkv, np.float32)
    b_qkv = np.asarray(b_qkv, np.float32)
    tri = (np.arange(P)[None, :] >= np.arange(P)[:, None]).astype(BF16)
    in_maps = []
    for c in range(NCORES):
        b, g = c // 2, c % 2
        xT = np.ascontiguousarray(x[b].T).astype(BF16)
        # interleaved [K_m | Q_m] 128-col pairs
        wkq = np.empty((D, 2 * GCOLS), np.float32)
        bqk = np.empty((P, 2 * NMC), np.float32)
        for m in range(NMC):
            qs = g * GCOLS + m * P
            ks = D + g * GCOLS + m * P
            wkq[:, 256 * m:256 * m + P] = w_qkv[:, ks:ks + P]
            wkq[:, 256 * m + P:256 * (m + 1)] = w_qkv[:, qs:qs + P]
            bqk[:, 2 * m] = b_qkv[ks:ks + P]
            bqk[:, 2 * m + 1] = b_qkv[qs:qs + P]
        wkq = wkq.astype(BF16)
        wv = np.ascontiguousarray(
            w_qkv[:, 2 * D + g * GCOLS: 2 * D + (g + 1) * GCOLS]).astype(BF16)
        bv = np.ascontiguousarray(
            b_qkv[2 * D + g * GCOLS: 2 * D + (g + 1) * GCOLS]).astype(np.float32)
        in_maps.append({
            "xT": xT, "wkq": wkq, "wv": wv,
            "wp": None,  # filled by caller (needs w_proj)
            "bqk": bqk, "bv": bv, "tri": tri,
        })
    return in_maps


def full_in_maps(x, w_qkv, b_qkv, w_proj):
    w_proj = np.asarray(w_proj, np.float32)
    in_maps = host_inputs(x, w_qkv, b_qkv)
    for c in range(NCORES):
        g = c % 2
        in_maps[c]["wp"] = np.ascontiguousarray(
            w_proj[g * GCOLS:(g + 1) * GCOLS, :]).astype(BF16)
    return in_maps


def gather(results, b_proj):
    out = np.zeros((B, T, D), np.float32)
    for c in range(NCORES):
        out[c // 2] += results[c]["outp"].astype(np.float32)
    out += np.asarray(b_proj, np.float32)[None, None, :]
    return out


_NC_CACHE = None


def kernel(x, w_qkv, b_qkv, w_proj, b_proj):
    global _NC_CACHE
    if _NC_CACHE is None:
        _NC_CACHE = build_nc()
    in_maps = full_in_maps(x, w_qkv, b_qkv, w_proj)
    res = run_bass_kernel_spmd(_NC_CACHE, in_maps, core_ids=list(range(NCORES)))
    return gather(res.results, b_proj)


if __name__ == "__main__":
    rng = np.random.default_rng(0)
    x = rng.standard_normal((B, T, D), dtype=np.float32)
    w_qkv = rng.standard_normal((D, 3 * D), dtype=np.float32) / np.sqrt(D)
    b_qkv = np.zeros(3 * D, np.float32)
    w_proj = rng.standard_normal((D, D), dtype=np.float32) / np.sqrt(D)
    b_proj = np.zeros(D, np.float32)
    out = kernel(x, w_qkv, b_qkv, w_proj, b_proj)
    print(out.shape, out.dtype)



# revision 52
# speedup vs baseline: 1.0353x; 1.0353x over previous
"""Causal self-attention Trainium2 kernel (v2).

Problem: B=4, T=2048, D=1024, H=16 heads (hd=64).
Sharding: 8 cores; core c -> batch c//2, heads (c%2)*8 .. +8.
Each core computes a partial output projection (its 512 rows of w_proj);
host sums the two partials per batch and adds b_proj.

v2 changes over the 247us baseline:
  - QKV projections run as fp8e4 DoubleRow 3-chain GEMMs
    (xh@wh + xh@wl + xl@wh, hi/lo splits prepared on host, w scaled by
    16 so the lo residual clears the fp8 denormal floor).  0.75x engine
    cycles AND 256-deep contraction per matmul.
  - P@V is flipped: out[q,65] = pts_block.T @ V_aug with N=65 columns
    per (q-block, k-block) instead of N=512 per k-block -- half the PE
    cycles.  The 8 per-group accumulators (2 heads x 4 q-subblocks)
    live in one [128,8,128]-padded PSUM tile (2 banks).  Denominators
    land per-partition, so normalization is a [128,2] reciprocal + one
    stride0-broadcast tensor_tensor -- no GpSimd partition_broadcast,
    no [1,512] reciprocals, no staging copies.
  - A is produced in natural [t, d] layout and PE-transposed (identity
    matmul, 53ns each) into A^T for the output projection lhsT.
  - Scale bookkeeping: Q,K,V carried at 16x (w scaled on host), scores
    at 256x (exp scale 0.125/256), P at 64x (exp bias ln 64, which also
    lifts softmax weights out of the bf16/fp8 denormal zone), A at 16x,
    out-proj product at 256x, divided back in the final PSUM->SBUF copy.

Schedule: same machinery as the baseline -- batched priority-ordered
input DMA ladder, deadline-ordered filler generators pumped between
attention steps, P@V emitted one ki late so the in-order PE queue never
waits on the Exp stream; normalize/transpose for q-subblock accumulators
emitted as soon as their last k-block lands (staggered across the ki
loop, transposes deferred one step).
"""

import math
import os
import sys
from collections import deque

PUMP_SAFE_N = int(os.environ.get("K_PUMP_SAFE", "3"))
PUMP_HI = int(os.environ.get("K_PUMP_HI", "3"))
PUMP_LO = int(os.environ.get("K_PUMP_LO", "1"))
PV_LATE = int(os.environ.get("K_PV_LATE", "2"))

for _p in ("/opt/trn_rl_repo",):
    if _p not in sys.path:
        sys.path.insert(0, _p)

import numpy as np
import ml_dtypes

import concourse.bass as bass
import concourse.mybir as mybir
import concourse.tile as tile
from concourse import bacc
from concourse.bass_utils import run_bass_kernel_spmd

BF16 = ml_dtypes.bfloat16
E4 = ml_dtypes.float8_e4m3fn

B, T, D = 4, 2048, 1024
H, HD = 16, 64
NCORES = 8
HPC = 8                  # heads per core
GCOLS = HPC * HD         # 512 columns of qkv per core per q/k/v
P = 128
NDC2 = 4                 # 4 DoubleRow contraction pairs of 256
NTT = T // P             # 16 t-tiles of 128
NQC = T // 512           # 4 q-chunks of 512
NMC = GCOLS // P         # 4 M-chunks per Q^T / K^T (2 heads each)

WS = 16.0                # host-side weight scale (w * 16)
EXP_SCALE = 0.125 / (WS * WS)      # scores carried at 256x
EXP_BIAS = math.log(64.0)          # P carried at 64x
OUT_SCALE = 1.0 / (WS * WS)        # A at 16x, wp at 16x -> /256

DR = mybir.MatmulPerfMode.DoubleRow


def _ap3(dram, off, part_stride, nparts, mid_stride, nmid, inner):
    """3-level DRAM access pattern: [partition, mid, contiguous-inner]."""
    a = dram.ap()
    return bass.AP(tensor=a.tensor, offset=a.offset + off,
                   ap=[[part_stride, nparts], [mid_stride, nmid], [1, inner]])


def build_nc(trace_sim: bool = False):
    f32 = mybir.dt.float32
    bf16 = mybir.dt.bfloat16
    fp8 = mybir.dt.float8e4

    nc = bacc.Bacc("TRN2", target_bir_lowering=False, debug=False,
                   num_devices=NCORES)

    # x^T hi/lo splits: [p, c, i, t] = split(x[t, 256c + 128i + p])
    xh_d = nc.dram_tensor("xh", [P, NDC2, 2, T], fp8, kind="ExternalInput")
    xl_d = nc.dram_tensor("xl", [P, NDC2, 2, T], fp8, kind="ExternalInput")
    # wkq hi/lo: [p, m, c, i, 256] -- per-m [K_m | Q_m] 128-col pairs,
    # rows regrouped into DoubleRow pairs, values scaled by 16
    wkqh_d = nc.dram_tensor("wkqh", [P, NMC, NDC2, 2, 256], fp8,
                            kind="ExternalInput")
    wkql_d = nc.dram_tensor("wkql", [P, NMC, NDC2, 2, 256], fp8,
                            kind="ExternalInput")
    wvh_d = nc.dram_tensor("wvh", [P, NDC2, 2, GCOLS], fp8,
                           kind="ExternalInput")
    wvl_d = nc.dram_tensor("wvl", [P, NDC2, 2, GCOLS], fp8,
                           kind="ExternalInput")
    wp_d = nc.dram_tensor("wp", [GCOLS, D], bf16, kind="ExternalInput")
    # bqk: col 2m = bias for K_m block, col 2m+1 = bias for Q_m block (16x)
    bqk_d = nc.dram_tensor("bqk", [P, 2 * NMC], f32, kind="ExternalInput")
    bv_d = nc.dram_tensor("bv", [GCOLS], f32, kind="ExternalInput")
    tri_d = nc.dram_tensor("tri", [P, P], bf16, kind="ExternalInput")
    eye_d = nc.dram_tensor("eye", [P, P], bf16, kind="ExternalInput")
    out_d = nc.dram_tensor("outp", [T, D], bf16, kind="ExternalOutput")

    with tile.TileContext(nc, trace_sim=trace_sim) as tc:
        with (
            tc.tile_pool(name="consts", bufs=1) as consts,
            tc.tile_pool(name="weights", bufs=1) as weights,
            tc.tile_pool(name="acts", bufs=1) as acts,
            tc.tile_pool(name="pt", bufs=6) as ptp,
            tc.tile_pool(name="norm", bufs=4) as normp,
            tc.tile_pool(name="outs", bufs=3) as outsp,
            tc.tile_pool(name="ps_mm", bufs=2, space="PSUM") as ps_mm,
            tc.tile_pool(name="ps_st", bufs=2, space="PSUM") as ps_st,
            tc.tile_pool(name="ps_po", bufs=1, space="PSUM") as ps_po,
        ):
            xh_sb = acts.tile([P, NDC2, 2, T], fp8)
            xl_sb = acts.tile([P, NDC2, 2, T], fp8)
            wkqh_sb = weights.tile([P, NMC, NDC2, 2, 256], fp8)
            wkql_sb = weights.tile([P, NMC, NDC2, 2, 256], fp8)
            wvh_sb = weights.tile([P, NDC2, 2, GCOLS], fp8)
            wvl_sb = weights.tile([P, NDC2, 2, GCOLS], fp8)
            wp_sb = weights.tile([P, NMC, D], bf16)

            # ---- input DMA ladders: hi tensors on the SP HWDGE queue,
            # lo tensors + wp on the Activation HWDGE queue, both in
            # consumption-priority order ----
            def dma_x(eng, xd, xs, t0, t1):
                # slice [:, :, :, t0:t1]; (c, i) merge to one stride-T dim
                eng.dma_start(
                    xs[:, :, :, t0:t1],
                    bass.AP(tensor=xd.ap().tensor,
                            offset=xd.ap().offset + t0,
                            ap=[[NDC2 * 2 * T, P], [T, NDC2 * 2],
                                [1, t1 - t0]]))

            def dma_wkq(eng, wd, ws, m):
                # per-m slice, contiguous 2KB per partition on both sides
                eng.dma_start(
                    ws[:, m],
                    bass.AP(tensor=wd.ap().tensor,
                            offset=wd.ap().offset + m * 2048,
                            ap=[[NMC * 2048, P], [1, 2048]]))

            # Four parallel queues. Pool SWDGE: small consts first (the
            # V bias-add needs bv_rep, the first diagonal mask needs tri),
            # then wv hi/lo + wp; SP: xh + wkqh; ACT: xl + wkql.
            # Pool SWDGE, ordered by first use: wv hi/lo (V chains), then
            # the consts (bias-add, masks, transposes), then wp (late).
            nc.gpsimd.dma_start(wvh_sb[:], wvh_d.ap())
            nc.gpsimd.dma_start(wvl_sb[:], wvl_d.ap())
            bv_rep = consts.tile([P, GCOLS], f32)
            bv_ap = bv_d.ap()
            nc.gpsimd.dma_start(
                bv_rep[:],
                bass.AP(tensor=bv_ap.tensor, offset=bv_ap.offset,
                        ap=[[0, P]] + list(bv_ap.ap)),
            )
            bqk_sb = consts.tile([P, 2 * NMC], f32)
            nc.gpsimd.dma_start(bqk_sb[:], bqk_d.ap())
            tri_sb = consts.tile([P, P], bf16)
            nc.gpsimd.dma_start(tri_sb[:], tri_d.ap())
            eye_sb = consts.tile([P, P], bf16)
            nc.gpsimd.dma_start(eye_sb[:], eye_d.ap())
            nc.gpsimd.dma_start(wp_sb[:, :, :],
                                _ap3(wp_d, 0, D, P, P * D, NMC, D))

            # SP / ACT HWDGE: x slices early (the DR projections burn
            # through bytes fast); wkq m1-3 only feed (m>=1, *) groups.
            dma_x(nc.sync, xh_d, xh_sb, 0, 512)
            dma_wkq(nc.sync, wkqh_d, wkqh_sb, 0)
            dma_x(nc.sync, xh_d, xh_sb, 512, 1024)
            dma_x(nc.sync, xh_d, xh_sb, 1024, 2048)
            for m in range(1, NMC):
                dma_wkq(nc.sync, wkqh_d, wkqh_sb, m)

            dma_x(nc.scalar, xl_d, xl_sb, 0, 512)
            dma_wkq(nc.scalar, wkql_d, wkql_sb, 0)
            dma_x(nc.scalar, xl_d, xl_sb, 512, 1024)
            dma_x(nc.scalar, xl_d, xl_sb, 1024, 2048)
            for m in range(1, NMC):
                dma_wkq(nc.scalar, wkql_d, wkql_sb, m)

            # per-partition exp bias (ln 64 -> P carried at 64x)
            ebias = consts.tile([P, 1], f32)
            nc.vector.memset(ebias[:], EXP_BIAS)

            # zeros for the K=1 PSUM-bank-clearing matmuls (start=True
            # zeroes a whole 2KB bank, so the 8 P@V accumulators sharing
            # two banks get one explicit zeroing matmul per bank instead;
            # fp8 DoubleRow so each costs 256 instead of 512 PE cycles)
            zz = consts.tile([1, 2, 640], fp8)
            nc.vector.memset(zz[:], 0.0)

            # warm the ScalarE Exp table during the startup DMA window
            warm = consts.tile([1, 1], f32)
            nc.vector.memset(warm[:], 0.0)
            nc.scalar.activation(warm[:], warm[:],
                                 mybir.ActivationFunctionType.Exp)

            # V natural + ones column: [128, tt, head, 65] (values at 16x)
            V_sb = acts.tile([P, NTT, HPC, HD + 1], bf16)
            nc.vector.memset(V_sb[:, :, :, HD], 1.0)

            QT_sb = acts.tile([P, NMC, T], bf16)
            KT_sb = acts.tile([P, NMC, T], bf16)
            AT_sb = acts.tile([P, NMC, T], bf16)

            # ---------------- filler generators ----------------
            def dr_chains(lhs_pair, rhs_pair):
                """(lhsT, rhs) fp8 tensors -> 3-chain DoubleRow schedule:
                yields (lhs_sb, rhs_sb, c, is_first, is_last)."""
                (lh, ll), (rh, rl) = lhs_pair, rhs_pair
                chains = [(lh, rh), (lh, rl), (ll, rh)]
                n = len(chains) * NDC2
                i = 0
                for ls, rs in chains:
                    for c in range(NDC2):
                        yield ls, rs, c, i == 0, i == n - 1
                        i += 1

            def gen_v(tt):
                pv = ps_mm.tile([P, 512], f32, tag="mm", name=f"pv{tt}")
                sched = list(dr_chains((xh_sb, xl_sb), (wvh_sb, wvl_sb)))
                for ls, rs, c, first, last in sched:
                    nc.tensor.matmul(
                        pv[:],
                        ls[:, c, :, tt * P:(tt + 1) * P],
                        rs[:, c, :, :],
                        start=first, stop=last, perf_mode=DR,
                    )
                    if not last:
                        yield
                nc.vector.tensor_tensor(
                    V_sb[:, tt, :, 0:HD],
                    pv[:].rearrange("p (h d) -> p h d", h=HPC),
                    bv_rep[:].rearrange("p (h d) -> p h d", h=HPC),
                    mybir.AluOpType.add,
                )

            def gen_kq(is_q, m, tc4):
                off = P if is_q else 0
                pq = ps_mm.tile([P, 512], f32, tag="mm",
                                name=f"p{'q' if is_q else 'k'}{m}_{tc4}")
                sched = list(dr_chains((wkqh_sb, wkql_sb), (xh_sb, xl_sb)))
                for ls, rs, c, first, last in sched:
                    nc.tensor.matmul(
                        pq[:],
                        ls[:, m, c, :, off:off + P],
                        rs[:, c, :, tc4 * 512:(tc4 + 1) * 512],
                        start=first, stop=last, perf_mode=DR,
                    )
                    if not last:
                        yield
                dst = QT_sb if is_q else KT_sb
                bcol = 2 * m + (1 if is_q else 0)
                nc.vector.tensor_scalar_add(
                    dst[:, m, tc4 * 512:(tc4 + 1) * 512],
                    pq[:], bqk_sb[:, bcol:bcol + 1],
                )

            def gen_out(tt, ncol, tail=False, deng=None):
                pp = ps_mm.tile([P, 512], f32, tag="mm",
                                name=f"pp{tt}_{ncol}")
                for hc in range(NMC):
                    nc.tensor.matmul(
                        pp[:],
                        AT_sb[:, hc, tt * P:(tt + 1) * P],
                        wp_sb[:, hc, ncol * 512:(ncol + 1) * 512],
                        start=(hc == 0), stop=(hc == NMC - 1),
                    )
                    if hc < NMC - 1:
                        yield
                ot = outsp.tile([P, 512], bf16, tag="ot", bufs=7)
                if tail:
                    nc.scalar.activation(ot[:], pp[:],
                                         mybir.ActivationFunctionType.Copy,
                                         scale=OUT_SCALE)
                else:
                    nc.vector.tensor_scalar(ot[:], pp[:], OUT_SCALE, None,
                                            mybir.AluOpType.mult)
                (deng or nc.sync).dma_start(
                    out_d[tt * P:(tt + 1) * P,
                          ncol * 512:(ncol + 1) * 512],
                    ot[:],
                )

            fillers = deque()   # [deadline, generator, started]

            def pump(n):
                while n > 0 and fillers:
                    ent = fillers[0]
                    try:
                        next(ent[1])
                        ent[2] = True
                    except StopIteration:
                        fillers.popleft()
                    n -= 1

            def pump_safe(n):
                """Advance only already-started chains (their PSUM slot is
                held; further matmuls have no new waits) -- safe to emit
                ahead of the scores matmuls without risking a new stall."""
                while n > 0 and fillers and fillers[0][2]:
                    try:
                        next(fillers[0][1])
                    except StopIteration:
                        fillers.popleft()
                    n -= 1

            def drain_until(deadline):
                while fillers and fillers[0][0] <= deadline:
                    for _ in fillers.popleft()[1]:
                        pass

            def run_gen(g):
                for _ in g:
                    pass

            # ---------------- startup compute ----------------
            for tt in range(4):
                run_gen(gen_v(tt))
            run_gen(gen_kq(False, 0, 0))
            run_gen(gen_kq(True, 0, 0))

            # deadline-ordered filler queue. Deadline = one group BEFORE
            # the (mch, qc) at whose start the output is first consumed,
            # so a slow pump can't leave the consuming group's first
            # scores matmul waiting on a just-drained projection chain.
            def prev_group(m, qc):
                # shift deadlines one group early so the consuming group's
                # first scores never wait on a just-drained chain -- but
                # not for m==0 (startup DMA still in flight there)
                if m == 0:
                    return (m, qc)
                return (m, qc - 1) if qc > 0 else (m - 1, NQC - 1)

            for m in range(NMC):
                for qc in range(NQC):
                    if m == 0 and qc == 0:
                        continue
                    dl = prev_group(m, qc)
                    fillers.append([dl, gen_kq(False, m, qc), False])
                    fillers.append([dl, gen_kq(True, m, qc), False])
                    if m == 0:
                        for tt in range(4 * qc, 4 * qc + 4):
                            fillers.append([dl, gen_v(tt), False])

            # ---------------- attention main loop ----------------
            def norm_qsub(pot, qsub):
                """Normalize accumulators (j=0,1) for q-subblock qsub into
                A natural layout; return the A_nat tile for transposition.
                pot slot order: 2*(qsub%2) + j."""
                s0 = 2 * (qsub % 2)
                rs = normp.tile([P, 2], f32, tag="rs")
                nc.vector.reciprocal(
                    rs[:], pot[:, s0:s0 + 2, HD:HD + 1])
                an = normp.tile([P, 2, HD], bf16, tag="an")
                rs_ap = rs[:]
                rs_b = bass.AP(
                    tensor=rs_ap.tensor, offset=rs_ap.offset,
                    ap=[list(rs_ap.ap[0]), [1, 2], [0, HD]])
                nc.vector.tensor_tensor(
                    an[:], pot[:, s0:s0 + 2, 0:HD], rs_b,
                    mybir.AluOpType.mult)
                return an

            def transpose_qsub(an, mch, qc, qsub):
                tr = ps_mm.tile([P, P], bf16, tag="mm", name="tr")
                nc.tensor.transpose(tr[:], an[:], eye_sb[:])
                nc.vector.tensor_copy(
                    AT_sb[:, mch, qc * 512 + qsub * P:
                          qc * 512 + (qsub + 1) * P], tr[:])

            for mch in range(NMC):
                for qc in range(NQC):
                    drain_until((mch, qc))
                    po = ps_po.tile([P, 8, P], f32, tag="po",
                                    name=f"po{mch}_{qc}")
                    po2 = [po[:, 0:4], po[:, 4:8]]

                    def zero_po(half):
                        # one start=True matmul per 2KB accumulator bank
                        # (start zeroes a whole bank), emitted just before
                        # the bank's first P@V
                        nc.tensor.matmul(
                            po[:, 4 * half:4 * half + 4, :],
                            zz[0:1, :, 0:P], zz[0:1, :, P:P + 512],
                            start=True, stop=True,
                            skip_group_check=True, perf_mode=DR,
                        )
                    nki = 4 * qc + 4
                    pv_pending = deque()
                    pending_tr = []
                    cur_ki = [0]
                    zeroed = [False, False]

                    def pop_pv():
                        ki_, pts_ = pv_pending.popleft()
                        for half in range(2):
                            if not zeroed[half]:
                                zero_po(half)
                                zeroed[half] = True
                        do_pv(ki_, pts_)

                    def do_pv(ki, pts):
                        # P@V for k-block ki into accumulators (two-late)
                        for qsub in range(4):
                            qlim = 4 * qc + qsub
                            if ki > qlim:
                                continue
                            for j in range(2):
                                nc.tensor.matmul(
                                    po2[qsub // 2][:, 2 * (qsub % 2) + j,
                                                   0:HD + 1],
                                    pts[:, j, qsub * P:(qsub + 1) * P],
                                    V_sb[:, ki, 2 * mch + j, :],
                                    start=False, stop=(ki == qlim),
                                    skip_group_check=True,
                                )
                        if ki >= 4 * qc:
                            qsub = ki - 4 * qc
                            an = norm_qsub(po2[qsub // 2], qsub)
                            pending_tr.append((cur_ki[0], an, mch, qc, qsub))

                    for ki in range(nki):
                        off = max(0, ki - 4 * qc) * P
                        # continue in-flight filler chains ahead of the
                        # scores matmuls (no new waits -> absorbs ACT lag
                        # without risking a fresh-chain stall)
                        pump_safe(PUMP_SAFE_N)
                        pshat = ps_st.tile([P, 2, 512], f32, tag="st")
                        pts = ptp.tile([P, 2, 512], bf16, tag="pt")
                        for j in range(2):
                            part = j * 64
                            nc.tensor.matmul(
                                pshat[:, j, off:512],
                                KT_sb[part:part + 64, mch,
                                      ki * P:(ki + 1) * P],
                                QT_sb[part:part + 64, mch,
                                      qc * 512 + off:(qc + 1) * 512],
                                start=True, stop=True,
                            )
                        nc.scalar.activation(
                            pts[:, :, off:512], pshat[:, :, off:512],
                            mybir.ActivationFunctionType.Exp,
                            scale=EXP_SCALE, bias=ebias[:],
                        )
                        if ki >= 4 * qc:
                            # diagonal block: zero out q < k entries.
                            # On GpSimd (SBUF-only engine, otherwise idle)
                            # so the exp->mask chain never backs up the
                            # DVE queue; tri broadcast over j via a
                            # 0-stride AP dim.
                            tri_ap = tri_sb[:]
                            tri_b = bass.AP(
                                tensor=tri_ap.tensor, offset=tri_ap.offset,
                                ap=[list(tri_ap.ap[0]), [0, 2], [1, P]])
                            nc.gpsimd.tensor_tensor(
                                pts[:, :, off:off + P],
                                pts[:, :, off:off + P],
                                tri_b, mybir.AluOpType.mult,
                            )
                        cur_ki[0] = ki
                        pump(PUMP_HI if mch == NMC - 1 else PUMP_LO)
                        # flush transposes whose normalize is >= 2 steps old
                        while pending_tr and pending_tr[0][0] <= ki - 2:
                            _, an_, m_, q_, s_ = pending_tr.pop(0)
                            transpose_qsub(an_, m_, q_, s_)
                            if mch == NMC - 1:
                                # mch 3: this row-block's A^T is complete,
                                # its out-proj becomes pump fodder now
                                for ncol in range(2):
                                    fillers.append(
                                        [(8, q_, s_),
                                         gen_out(4 * q_ + s_, ncol), False])
                        pv_pending.append((ki, pts))
                        if len(pv_pending) > PV_LATE:
                            pop_pv()
                    # remaining k-blocks (two-late, post loop)
                    while pv_pending:
                        pop_pv()
                    while pending_tr:
                        _, an_, m_, q_, s_ = pending_tr.pop(0)
                        transpose_qsub(an_, m_, q_, s_)
                        if mch == NMC - 1:
                            last = qc == NQC - 1
                            for ncol in range(2):
                                fillers.append(
                                    [(8, q_, s_),
                                     gen_out(4 * q_ + s_, ncol,
                                             tail=(last and ncol == 1),
                                             deng=(nc.scalar
                                                   if last and ncol
                                                   else None)), False])
            # ---- tail: drain remaining fillers round-robin ----
            wave = deque(e[1] for e in fillers)
            while wave:
                g = wave.popleft()
                try:
                    next(g)
                except StopIteration:
                    continue
                wave.append(g)

    nc.compile()
    return nc


def _split8(a):
    """fp32 -> (hi, lo) e4m3 split."""
    hi = a.astype(E4)
    lo = (a - hi.astype(np.float32)).astype(E4)
    return hi, lo


def host_inputs(x, w_qkv, b_qkv):
    """Per-core input maps. Core c -> batch c//2, head group c%2."""
    x = np.asarray(x, np.float32)
    w_qkv = np.asarray(w_qkv, np.float32) * WS
    b_qkv = np.asarray(b_qkv, np.float32) * WS
    tri = (np.arange(P)[None, :] >= np.arange(P)[:, None]).astype(BF16)
    eye = np.eye(P, dtype=np.float32).astype(BF16)

    # x^T regrouped to DoubleRow pairs: [p, c, i, t] = x[t, 256c+128i+p]
    xp_by_batch = []
    for b in range(B):
        xT = np.ascontiguousarray(x[b].T)          # [D, T]
        xr = xT.reshape(NDC2, 2, P, T).transpose(2, 0, 1, 3)  # [p,c,i,t]
        xp_by_batch.append(_split8(np.ascontiguousarray(xr)))

    wkq_by_g = []
    wv_by_g = []
    bqk_by_g = []
    bv_by_g = []
    for g in range(2):
        # interleaved [K_m | Q_m] 128-col pairs
        wkq = np.empty((D, 2 * GCOLS), np.float32)
        bqk = np.empty((P, 2 * NMC), np.float32)
        for m in range(NMC):
            qs = g * GCOLS + m * P
            ks = D + g * GCOLS + m * P
            wkq[:, 256 * m:256 * m + P] = w_qkv[:, ks:ks + P]
            wkq[:, 256 * m + P:256 * (m + 1)] = w_qkv[:, qs:qs + P]
            bqk[:, 2 * m] = b_qkv[ks:ks + P]
            bqk[:, 2 * m + 1] = b_qkv[qs:qs + P]
        # [d, col] -> [p, m, c, i, 256]: d = 256c+128i+p, col = 256m+w
        wkqr = wkq.reshape(NDC2, 2, P, NMC, 256).transpose(2, 3, 0, 1, 4)
        wkq_by_g.append(_split8(np.ascontiguousarray(wkqr)))
        wv = np.ascontiguousarray(
            w_qkv[:, 2 * D + g * GCOLS: 2 * D + (g + 1) * GCOLS])
        wvr = wv.reshape(NDC2, 2, P, GCOLS).transpose(2, 0, 1, 3)
        wv_by_g.append(_split8(np.ascontiguousarray(wvr)))
        bqk_by_g.append(bqk)
        bv_by_g.append(np.ascontiguousarray(
            b_qkv[2 * D + g * GCOLS: 2 * D + (g + 1) * GCOLS]).astype(
                np.float32))

    in_maps = []
    for c in range(NCORES):
        b, g = c // 2, c % 2
        xh, xl = xp_by_batch[b]
        wkqh, wkql = wkq_by_g[g]
        wvh, wvl = wv_by_g[g]
        in_maps.append({
            "xh": xh, "xl": xl,
            "wkqh": wkqh, "wkql": wkql,
            "wvh": wvh, "wvl": wvl,
            "wp": None,  # filled by caller (needs w_proj)
            "bqk": bqk_by_g[g], "bv": bv_by_g[g],
            "tri": tri, "eye": eye,
        })
    return in_maps


def full_in_maps(x, w_qkv, b_qkv, w_proj):
    w_proj = np.asarray(w_proj, np.float32) * WS
    in_maps = host_inputs(x, w_qkv, b_qkv)
    for c in range(NCORES):
        g = c % 2
        in_maps[c]["wp"] = np.ascontiguousarray(
            w_proj[g * GCOLS:(g + 1) * GCOLS, :]).astype(BF16)
    return in_maps


def gather(results, b_proj):
    out = np.zeros((B, T, D), np.float32)
    for c in range(NCORES):
        out[c // 2] += results[c]["outp"].astype(np.float32)
    out += np.asarray(b_proj, np.float32)[None, None, :]
    return out


_NC_CACHE = None


def kernel(x, w_qkv, b_qkv, w_proj, b_proj):
    global _NC_CACHE
    if _NC_CACHE is None:
        _NC_CACHE = build_nc()
    in_maps = full_in_maps(x, w_qkv, b_qkv, w_proj)
    res = run_bass_kernel_spmd(_NC_CACHE, in_maps, core_ids=list(range(NCORES)))
    return gather(res.results, b_proj)


if __name__ == "__main__":
    rng = np.random.default_rng(0)
    x = rng.standard_normal((B, T, D), dtype=np.float32)
    w_qkv = rng.standard_normal((D, 3 * D), dtype=np.float32) / np.sqrt(D)
    b_qkv = np.zeros(3 * D, np.float32)
    w_proj = rng.standard_normal((D, D), dtype=np.float32) / np.sqrt(D)
    b_proj = np.zeros(D, np.float32)
    out = kernel(x, w_qkv, b_qkv, w_proj, b_proj)
    print(out.shape, out.dtype)


# revision 58
# speedup vs baseline: 1.0768x; 1.0401x over previous
"""Causal self-attention Trainium2 kernel (v2).

Problem: B=4, T=2048, D=1024, H=16 heads (hd=64).
Sharding: 8 cores; core c -> batch c//2, heads (c%2)*8 .. +8.
Each core computes a partial output projection (its 512 rows of w_proj);
host sums the two partials per batch and adds b_proj.

v2 changes over the 247us baseline:
  - QKV projections run as fp8e4 DoubleRow 3-chain GEMMs
    (xh@wh + xh@wl + xl@wh, hi/lo splits prepared on host, w scaled by
    16 so the lo residual clears the fp8 denormal floor).  0.75x engine
    cycles AND 256-deep contraction per matmul.
  - P@V is flipped: out[q,65] = pts_block.T @ V_aug with N=65 columns
    per (q-block, k-block) instead of N=512 per k-block -- half the PE
    cycles.  The 8 per-group accumulators (2 heads x 4 q-subblocks)
    live in one [128,8,128]-padded PSUM tile (2 banks).  Denominators
    land per-partition, so normalization is a [128,2] reciprocal + one
    stride0-broadcast tensor_tensor -- no GpSimd partition_broadcast,
    no [1,512] reciprocals, no staging copies.
  - A is produced in natural [t, d] layout and PE-transposed (identity
    matmul, 53ns each) into A^T for the output projection lhsT.
  - Scale bookkeeping: Q,K,V carried at 16x (w scaled on host), scores
    at 256x (exp scale 0.125/256), P at 64x (exp bias ln 64, which also
    lifts softmax weights out of the bf16/fp8 denormal zone), A at 16x,
    out-proj product at 256x, divided back in the final PSUM->SBUF copy.

Schedule: same machinery as the baseline -- batched priority-ordered
input DMA ladder, deadline-ordered filler generators pumped between
attention steps, P@V emitted one ki late so the in-order PE queue never
waits on the Exp stream; normalize/transpose for q-subblock accumulators
emitted as soon as their last k-block lands (staggered across the ki
loop, transposes deferred one step).
"""

import math
import os
import sys
from collections import deque

PUMP_SAFE_N = int(os.environ.get("K_PUMP_SAFE", "3"))
PUMP_HI = int(os.environ.get("K_PUMP_HI", "3"))
PUMP_LO = int(os.environ.get("K_PUMP_LO", "1"))
PV_LATE = int(os.environ.get("K_PV_LATE", "2"))

for _p in ("/opt/trn_rl_repo",):
    if _p not in sys.path:
        sys.path.insert(0, _p)

import numpy as np
import ml_dtypes

import concourse.bass as bass
import concourse.mybir as mybir
import concourse.tile as tile
from concourse import bacc
from concourse.bass_utils import run_bass_kernel_spmd

BF16 = ml_dtypes.bfloat16
E4 = ml_dtypes.float8_e4m3fn

B, T, D = 4, 2048, 1024
H, HD = 16, 64
NCORES = 8
HPC = 8                  # heads per core
GCOLS = HPC * HD         # 512 columns of qkv per core per q/k/v
P = 128
NDC2 = 4                 # 4 DoubleRow contraction pairs of 256
NTT = T // P             # 16 t-tiles of 128
NQC = T // 512           # 4 q-chunks of 512
NMC = GCOLS // P         # 4 M-chunks per Q^T / K^T (2 heads each)

WS = 16.0                # host-side weight scale (w * 16)
EXP_SCALE = 0.125 / (WS * WS)      # scores carried at 256x
EXP_BIAS = math.log(64.0)          # P carried at 64x
OUT_SCALE = 1.0 / (WS * WS)        # A at 16x, wp at 16x -> /256

DR = mybir.MatmulPerfMode.DoubleRow


def _ap3(dram, off, part_stride, nparts, mid_stride, nmid, inner):
    """3-level DRAM access pattern: [partition, mid, contiguous-inner]."""
    a = dram.ap()
    return bass.AP(tensor=a.tensor, offset=a.offset + off,
                   ap=[[part_stride, nparts], [mid_stride, nmid], [1, inner]])


def build_nc(trace_sim: bool = False):
    f32 = mybir.dt.float32
    bf16 = mybir.dt.bfloat16
    fp8 = mybir.dt.float8e4

    nc = bacc.Bacc("TRN2", target_bir_lowering=False, debug=False,
                   num_devices=NCORES)

    # x^T hi/lo splits: [p, c, i, t] = split(x[t, 256c + 128i + p])
    xh_d = nc.dram_tensor("xh", [P, NDC2, 2, T], fp8, kind="ExternalInput")
    xl_d = nc.dram_tensor("xl", [P, NDC2, 2, T], fp8, kind="ExternalInput")
    # wkq hi/lo: [p, m, c, i, 256] -- per-m [K_m | Q_m] 128-col pairs,
    # rows regrouped into DoubleRow pairs, values scaled by 16
    wkqh_d = nc.dram_tensor("wkqh", [P, NMC, NDC2, 2, 256], fp8,
                            kind="ExternalInput")
    wkql_d = nc.dram_tensor("wkql", [P, NMC, NDC2, 2, 256], fp8,
                            kind="ExternalInput")
    wvh_d = nc.dram_tensor("wvh", [P, NDC2, 2, GCOLS], fp8,
                           kind="ExternalInput")
    wvl_d = nc.dram_tensor("wvl", [P, NDC2, 2, GCOLS], fp8,
                           kind="ExternalInput")
    wp_d = nc.dram_tensor("wp", [GCOLS, D], bf16, kind="ExternalInput")
    # bqk: col 2m = bias for K_m block, col 2m+1 = bias for Q_m block (16x)
    bqk_d = nc.dram_tensor("bqk", [P, 2 * NMC], f32, kind="ExternalInput")
    bv_d = nc.dram_tensor("bv", [GCOLS], f32, kind="ExternalInput")
    tri_d = nc.dram_tensor("tri", [P, P], bf16, kind="ExternalInput")
    eye_d = nc.dram_tensor("eye", [P, P], bf16, kind="ExternalInput")
    out_d = nc.dram_tensor("outp", [T, D], bf16, kind="ExternalOutput")

    with tile.TileContext(nc, trace_sim=trace_sim) as tc:
        with (
            tc.tile_pool(name="consts", bufs=1) as consts,
            tc.tile_pool(name="weights", bufs=1) as weights,
            tc.tile_pool(name="acts", bufs=1) as acts,
            tc.tile_pool(name="pt", bufs=6) as ptp,
            tc.tile_pool(name="norm", bufs=4) as normp,
            tc.tile_pool(name="outs", bufs=3) as outsp,
            tc.tile_pool(name="ps_mm", bufs=2, space="PSUM") as ps_mm,
            tc.tile_pool(name="ps_st", bufs=2, space="PSUM") as ps_st,
            tc.tile_pool(name="ps_po", bufs=1, space="PSUM") as ps_po,
            tc.tile_pool(name="ps_pb", bufs=1, space="PSUM") as ps_pb,
        ):
            xh_sb = acts.tile([P, NDC2, 2, T], fp8)
            xl_sb = acts.tile([P, NDC2, 2, T], fp8)
            wkqh_sb = weights.tile([P, NMC, NDC2, 2, 256], fp8)
            wkql_sb = weights.tile([P, NMC, NDC2, 2, 256], fp8)
            wvh_sb = weights.tile([P, NDC2, 2, GCOLS], fp8)
            wvl_sb = weights.tile([P, NDC2, 2, GCOLS], fp8)
            wp_sb = weights.tile([P, NMC, D], bf16)

            # ---- input DMA ladders: hi tensors on the SP HWDGE queue,
            # lo tensors + wp on the Activation HWDGE queue, both in
            # consumption-priority order ----
            def dma_x(eng, xd, xs, t0, t1):
                # slice [:, :, :, t0:t1]; (c, i) merge to one stride-T dim
                eng.dma_start(
                    xs[:, :, :, t0:t1],
                    bass.AP(tensor=xd.ap().tensor,
                            offset=xd.ap().offset + t0,
                            ap=[[NDC2 * 2 * T, P], [T, NDC2 * 2],
                                [1, t1 - t0]]))

            def dma_wkq(eng, wd, ws, m):
                # per-m slice, contiguous 2KB per partition on both sides
                eng.dma_start(
                    ws[:, m],
                    bass.AP(tensor=wd.ap().tensor,
                            offset=wd.ap().offset + m * 2048,
                            ap=[[NMC * 2048, P], [1, 2048]]))

            # Four parallel queues. Pool SWDGE: small consts first (the
            # V bias-add needs bv_rep, the first diagonal mask needs tri),
            # then wv hi/lo + wp; SP: xh + wkqh; ACT: xl + wkql.
            # Pool SWDGE, ordered by first use: wv hi/lo (V chains), then
            # the consts (bias-add, masks, transposes), then wp (late).
            nc.gpsimd.dma_start(wvh_sb[:], wvh_d.ap())
            bv_rep = consts.tile([P, GCOLS], f32)
            bv_ap = bv_d.ap()
            nc.gpsimd.dma_start(
                bv_rep[:],
                bass.AP(tensor=bv_ap.tensor, offset=bv_ap.offset,
                        ap=[[0, P]] + list(bv_ap.ap)),
            )
            bqk_sb = consts.tile([P, 2 * NMC], f32)
            nc.gpsimd.dma_start(bqk_sb[:], bqk_d.ap())
            tri_sb = consts.tile([P, P], bf16)
            nc.gpsimd.dma_start(tri_sb[:], tri_d.ap())
            eye_sb = consts.tile([P, P], bf16)
            nc.gpsimd.dma_start(eye_sb[:], eye_d.ap())
            nc.gpsimd.dma_start(wp_sb[:, :, :],
                                _ap3(wp_d, 0, D, P, P * D, NMC, D))

            # SP / ACT HWDGE: x slices early (the DR projections burn
            # through bytes fast); wkq m1-3 only feed (m>=1, *) groups.
            dma_x(nc.sync, xh_d, xh_sb, 0, 512)
            dma_wkq(nc.sync, wkqh_d, wkqh_sb, 0)
            dma_x(nc.sync, xh_d, xh_sb, 512, 1024)
            dma_x(nc.sync, xh_d, xh_sb, 1024, 2048)
            for m in range(1, NMC):
                dma_wkq(nc.sync, wkqh_d, wkqh_sb, m)

            nc.scalar.dma_start(wvl_sb[:], wvl_d.ap())
            dma_x(nc.scalar, xl_d, xl_sb, 0, 512)
            dma_wkq(nc.scalar, wkql_d, wkql_sb, 0)
            dma_x(nc.scalar, xl_d, xl_sb, 512, 1024)
            dma_x(nc.scalar, xl_d, xl_sb, 1024, 2048)
            for m in range(1, NMC):
                dma_wkq(nc.scalar, wkql_d, wkql_sb, m)

            # per-partition exp bias (ln 64 -> P carried at 64x)
            ebias = consts.tile([P, 1], f32)
            nc.vector.memset(ebias[:], EXP_BIAS)

            # zeros for the K=1 PSUM-bank-clearing matmuls (start=True
            # zeroes a whole 2KB bank, so the 8 P@V accumulators sharing
            # two banks get one explicit zeroing matmul per bank instead;
            # fp8 DoubleRow so each costs 256 instead of 512 PE cycles)
            zz = consts.tile([1, 2, 640], fp8)
            nc.vector.memset(zz[:], 0.0)

            # warm the ScalarE Exp table during the startup DMA window
            warm = consts.tile([1, 1], f32)
            nc.vector.memset(warm[:], 0.0)
            nc.scalar.activation(warm[:], warm[:],
                                 mybir.ActivationFunctionType.Exp)

            # V natural + ones column: [128, tt, head, 65] (values at 16x)
            V_sb = acts.tile([P, NTT, HPC, HD + 1], bf16)
            nc.vector.memset(V_sb[:, :, :, HD], 1.0)

            QT_sb = acts.tile([P, NMC, T], bf16)
            KT_sb = acts.tile([P, NMC, T], bf16)
            AT_sb = acts.tile([P, NMC, T], bf16)

            # ---------------- filler generators ----------------
            def dr_chains(lhs_pair, rhs_pair):
                """(lhsT, rhs) fp8 tensors -> 3-chain DoubleRow schedule:
                yields (lhs_sb, rhs_sb, c, is_first, is_last)."""
                (lh, ll), (rh, rl) = lhs_pair, rhs_pair
                chains = [(lh, rh), (lh, rl), (ll, rh)]
                n = len(chains) * NDC2
                i = 0
                for ls, rs in chains:
                    for c in range(NDC2):
                        yield ls, rs, c, i == 0, i == n - 1
                        i += 1

            def gen_v(tt):
                pv = ps_mm.tile([P, 512], f32, tag="mm", name=f"pv{tt}")
                sched = list(dr_chains((xh_sb, xl_sb), (wvh_sb, wvl_sb)))
                for ls, rs, c, first, last in sched:
                    nc.tensor.matmul(
                        pv[:],
                        ls[:, c, :, tt * P:(tt + 1) * P],
                        rs[:, c, :, :],
                        start=first, stop=last, perf_mode=DR,
                    )
                    if not last:
                        yield
                nc.vector.tensor_tensor(
                    V_sb[:, tt, :, 0:HD],
                    pv[:].rearrange("p (h d) -> p h d", h=HPC),
                    bv_rep[:].rearrange("p (h d) -> p h d", h=HPC),
                    mybir.AluOpType.add,
                )

            def gen_kq(is_q, m, tc4):
                off = P if is_q else 0
                pq = ps_mm.tile([P, 512], f32, tag="mm",
                                name=f"p{'q' if is_q else 'k'}{m}_{tc4}")
                sched = list(dr_chains((wkqh_sb, wkql_sb), (xh_sb, xl_sb)))
                for ls, rs, c, first, last in sched:
                    nc.tensor.matmul(
                        pq[:],
                        ls[:, m, c, :, off:off + P],
                        rs[:, c, :, tc4 * 512:(tc4 + 1) * 512],
                        start=first, stop=last, perf_mode=DR,
                    )
                    if not last:
                        yield
                dst = QT_sb if is_q else KT_sb
                bcol = 2 * m + (1 if is_q else 0)
                nc.vector.tensor_scalar_add(
                    dst[:, m, tc4 * 512:(tc4 + 1) * 512],
                    pq[:], bqk_sb[:, bcol:bcol + 1],
                )

            at_ready = set()

            def gen_out(tt, ncol, tail=False, deng=None, eager=False):
                pp = ps_mm.tile([P, 512], f32, tag="mm",
                                name=f"pp{tt}_{ncol}")
                for hc in range(NMC):
                    if eager and hc == NMC - 1:
                        # final chunk reads AT[:, 3, tt]: spin until that
                        # A^T block's copy has been EMITTED (program order
                        # is the only ordering the tile deps can see)
                        while tt not in at_ready:
                            yield
                    nc.tensor.matmul(
                        pp[:],
                        AT_sb[:, hc, tt * P:(tt + 1) * P],
                        wp_sb[:, hc, ncol * 512:(ncol + 1) * 512],
                        start=(hc == 0), stop=(hc == NMC - 1),
                    )
                    if hc < NMC - 1:
                        yield
                ot = outsp.tile([P, 512], bf16, tag="ot", bufs=7)
                if tail:
                    nc.scalar.activation(ot[:], pp[:],
                                         mybir.ActivationFunctionType.Copy,
                                         scale=OUT_SCALE)
                else:
                    nc.vector.tensor_scalar(ot[:], pp[:], OUT_SCALE, None,
                                            mybir.AluOpType.mult)
                (deng or nc.sync).dma_start(
                    out_d[tt * P:(tt + 1) * P,
                          ncol * 512:(ncol + 1) * 512],
                    ot[:],
                )

            fillers = deque()   # [deadline, generator, started]

            def pump(n):
                while n > 0 and fillers:
                    ent = fillers[0]
                    try:
                        next(ent[1])
                        ent[2] = True
                    except StopIteration:
                        fillers.popleft()
                    n -= 1

            def pump_safe(n):
                """Advance only already-started chains (their PSUM slot is
                held; further matmuls have no new waits) -- safe to emit
                ahead of the scores matmuls without risking a new stall."""
                while n > 0 and fillers and fillers[0][2]:
                    try:
                        next(fillers[0][1])
                    except StopIteration:
                        fillers.popleft()
                    n -= 1

            def drain_until(deadline):
                while fillers and fillers[0][0] <= deadline:
                    for _ in fillers.popleft()[1]:
                        pass

            def run_gen(g):
                for _ in g:
                    pass

            # ---------------- startup compute ----------------
            for tt in range(4):
                run_gen(gen_v(tt))
            run_gen(gen_kq(False, 0, 0))
            run_gen(gen_kq(True, 0, 0))

            # deadline-ordered filler queue. Deadline = one group BEFORE
            # the (mch, qc) at whose start the output is first consumed,
            # so a slow pump can't leave the consuming group's first
            # scores matmul waiting on a just-drained projection chain.
            def prev_group(m, qc):
                # shift deadlines one group early so the consuming group's
                # first scores never wait on a just-drained chain -- but
                # not for m==0 (startup DMA still in flight there)
                if m == 0:
                    return (m, qc)
                return (m, qc - 1) if qc > 0 else (m - 1, NQC - 1)

            for m in range(NMC):
                for qc in range(NQC):
                    if m == 0 and qc == 0:
                        continue
                    dl = prev_group(m, qc)
                    fillers.append([dl, gen_kq(False, m, qc), False])
                    fillers.append([dl, gen_kq(True, m, qc), False])
                    if m == 0:
                        for tt in range(4 * qc, 4 * qc + 4):
                            fillers.append([dl, gen_v(tt), False])

            # ---------------- attention main loop ----------------
            def norm_qsub(pot, qsub):
                """Normalize accumulators (j=0,1) for q-subblock qsub into
                A natural layout; return the A_nat tile for transposition.
                pot slot order: 2*(qsub%2) + j."""
                s0 = 2 * (qsub % 2)
                rs = normp.tile([P, 2], f32, tag="rs")
                nc.vector.reciprocal(
                    rs[:], pot[:, s0:s0 + 2, HD:HD + 1])
                an = normp.tile([P, 2, HD], bf16, tag="an")
                rs_ap = rs[:]
                rs_b = bass.AP(
                    tensor=rs_ap.tensor, offset=rs_ap.offset,
                    ap=[list(rs_ap.ap[0]), [1, 2], [0, HD]])
                nc.vector.tensor_tensor(
                    an[:], pot[:, s0:s0 + 2, 0:HD], rs_b,
                    mybir.AluOpType.mult)
                return an

            def transpose_qsub(an, mch, qc, qsub):
                tr = ps_mm.tile([P, P], bf16, tag="mm", name="tr")
                nc.tensor.transpose(tr[:], an[:], eye_sb[:])
                nc.vector.tensor_copy(
                    AT_sb[:, mch, qc * 512 + qsub * P:
                          qc * 512 + (qsub + 1) * P], tr[:])
                if mch == NMC - 1:
                    at_ready.add(4 * qc + qsub)

            for mch in range(NMC):
                for qc in range(NQC):
                    drain_until((mch, qc))
                    # separate 1-bank tiles (separate pools): the next
                    # group's qsub0/1 tile only WARs against qsub0/1
                    # normalize reads, which happen early in this group
                    poA = ps_po.tile([P, 4, P], f32, tag="poA",
                                     name=f"poA{mch}_{qc}")
                    poB = ps_pb.tile([P, 4, P], f32, tag="poB",
                                     name=f"poB{mch}_{qc}")
                    po2 = [poA, poB]

                    def zero_po(half):
                        # one start=True matmul per 2KB accumulator bank
                        # (start zeroes a whole bank), emitted just before
                        # the bank's first P@V
                        nc.tensor.matmul(
                            po2[half][:, :, :],
                            zz[0:1, :, 0:P], zz[0:1, :, P:P + 512],
                            start=True, stop=True,
                            skip_group_check=True, perf_mode=DR,
                        )
                    if (mch, qc) == (NMC - 1, NQC - 1):
                        for s_ in range(4):
                            for ncol in range(2):
                                fillers.append(
                                    [(8, qc, s_),
                                     gen_out(12 + s_, ncol,
                                             tail=(ncol == 1),
                                             deng=(nc.scalar if ncol
                                                   else nc.sync),
                                             eager=True), False])
                    nki = 4 * qc + 4
                    pv_pending = deque()
                    pending_tr = []
                    deferred = []
                    cur_ki = [0]
                    zeroed = [False, False]

                    def pop_pv():
                        ki_, pts_ = pv_pending.popleft()
                        for half in range(2):
                            if not zeroed[half]:
                                zero_po(half)
                                zeroed[half] = True
                        do_pv(ki_, pts_)

                    def do_pv(ki, pts):
                        # P@V for k-block ki into accumulators (two-late)
                        for qsub in range(4):
                            qlim = 4 * qc + qsub
                            if ki > qlim:
                                continue
                            for j in range(2):
                                nc.tensor.matmul(
                                    po2[qsub // 2][:, 2 * (qsub % 2) + j,
                                                   0:HD + 1],
                                    pts[:, j, qsub * P:(qsub + 1) * P],
                                    V_sb[:, ki, 2 * mch + j, :],
                                    start=False, stop=(ki == qlim),
                                    skip_group_check=True,
                                )
                        if ki >= 4 * qc:
                            qsub = ki - 4 * qc
                            an = norm_qsub(po2[qsub // 2], qsub)
                            pending_tr.append((cur_ki[0], an, mch, qc, qsub))

                    for ki in range(nki):
                        off = max(0, ki - 4 * qc) * P
                        while deferred and deferred[0][0] <= ki:
                            fillers.append(deferred.pop(0)[1])
                        # continue in-flight filler chains ahead of the
                        # scores matmuls (no new waits -> absorbs ACT lag
                        # without risking a fresh-chain stall)
                        pump_safe(PUMP_SAFE_N)
                        pshat = ps_st.tile([P, 2, 512], f32, tag="st")
                        pts = ptp.tile([P, 2, 512], bf16, tag="pt")
                        for j in range(2):
                            part = j * 64
                            nc.tensor.matmul(
                                pshat[:, j, off:512],
                                KT_sb[part:part + 64, mch,
                                      ki * P:(ki + 1) * P],
                                QT_sb[part:part + 64, mch,
                                      qc * 512 + off:(qc + 1) * 512],
                                start=True, stop=True,
                            )
                        nc.scalar.activation(
                            pts[:, :, off:512], pshat[:, :, off:512],
                            mybir.ActivationFunctionType.Exp,
                            scale=EXP_SCALE, bias=ebias[:],
                        )
                        if ki >= 4 * qc:
                            # diagonal block: zero out q < k entries.
                            # On GpSimd (SBUF-only engine, otherwise idle)
                            # so the exp->mask chain never backs up the
                            # DVE queue; tri broadcast over j via a
                            # 0-stride AP dim.
                            tri_ap = tri_sb[:]
                            tri_b = bass.AP(
                                tensor=tri_ap.tensor, offset=tri_ap.offset,
                                ap=[list(tri_ap.ap[0]), [0, 2], [1, P]])
                            nc.vector.tensor_tensor(
                                pts[:, :, off:off + P],
                                pts[:, :, off:off + P],
                                tri_b, mybir.AluOpType.mult,
                            )
                        cur_ki[0] = ki
                        pump(8 if (mch, qc) == (NMC - 1, NQC - 1) else (PUMP_HI if mch == NMC - 1 else PUMP_LO))
                        # flush transposes whose normalize is >= 2 steps old
                        while pending_tr and pending_tr[0][0] <= ki - (1 if (mch, qc) == (NMC - 1, NQC - 1) else 3):
                            _, an_, m_, q_, s_ = pending_tr.pop(0)
                            transpose_qsub(an_, m_, q_, s_)
                            if mch == NMC - 1 and qc < NQC - 1:
                                # mch 3: this row-block's A^T is complete;
                                # its out-proj becomes pump fodder after a
                                # 2-iteration grace for the DVE copy
                                for ncol in range(2):
                                    deferred.append(
                                        (cur_ki[0] + 2,
                                         [(8, q_, s_),
                                          gen_out(4 * q_ + s_, ncol),
                                          False]))
                        pv_pending.append((ki, pts))
                        if len(pv_pending) > (1 if (mch, qc) == (NMC - 1, NQC - 1) else PV_LATE):
                            pop_pv()
                    # remaining k-blocks (two-late, post loop)
                    while pv_pending:
                        pop_pv()
                    while deferred:
                        fillers.append(deferred.pop(0)[1])
                    while pending_tr:
                        _, an_, m_, q_, s_ = pending_tr.pop(0)
                        transpose_qsub(an_, m_, q_, s_)
                        if mch == NMC - 1 and qc < NQC - 1:
                            for ncol in range(2):
                                fillers.append(
                                    [(8, q_, s_),
                                     gen_out(4 * q_ + s_, ncol), False])
            # ---- tail: drain remaining fillers round-robin ----
            wave = deque(e[1] for e in fillers)
            while wave:
                g = wave.popleft()
                try:
                    next(g)
                except StopIteration:
                    continue
                wave.append(g)

    nc.compile()
    return nc


def _split8(a):
    """fp32 -> (hi, lo) e4m3 split."""
    hi = a.astype(E4)
    lo = (a - hi.astype(np.float32)).astype(E4)
    return hi, lo


def host_inputs(x, w_qkv, b_qkv):
    """Per-core input maps. Core c -> batch c//2, head group c%2."""
    x = np.asarray(x, np.float32)
    w_qkv = np.asarray(w_qkv, np.float32) * WS
    b_qkv = np.asarray(b_qkv, np.float32) * WS
    tri = (np.arange(P)[None, :] >= np.arange(P)[:, None]).astype(BF16)
    eye = np.eye(P, dtype=np.float32).astype(BF16)

    # x^T regrouped to DoubleRow pairs: [p, c, i, t] = x[t, 256c+128i+p]
    xp_by_batch = []
    for b in range(B):
        xT = np.ascontiguousarray(x[b].T)          # [D, T]
        xr = xT.reshape(NDC2, 2, P, T).transpose(2, 0, 1, 3)  # [p,c,i,t]
        xp_by_batch.append(_split8(np.ascontiguousarray(xr)))

    wkq_by_g = []
    wv_by_g = []
    bqk_by_g = []
    bv_by_g = []
    for g in range(2):
        # interleaved [K_m | Q_m] 128-col pairs
        wkq = np.empty((D, 2 * GCOLS), np.float32)
        bqk = np.empty((P, 2 * NMC), np.float32)
        for m in range(NMC):
            qs = g * GCOLS + m * P
            ks = D + g * GCOLS + m * P
            wkq[:, 256 * m:256 * m + P] = w_qkv[:, ks:ks + P]
            wkq[:, 256 * m + P:256 * (m + 1)] = w_qkv[:, qs:qs + P]
            bqk[:, 2 * m] = b_qkv[ks:ks + P]
            bqk[:, 2 * m + 1] = b_qkv[qs:qs + P]
        # [d, col] -> [p, m, c, i, 256]: d = 256c+128i+p, col = 256m+w
        wkqr = wkq.reshape(NDC2, 2, P, NMC, 256).transpose(2, 3, 0, 1, 4)
        wkq_by_g.append(_split8(np.ascontiguousarray(wkqr)))
        wv = np.ascontiguousarray(
            w_qkv[:, 2 * D + g * GCOLS: 2 * D + (g + 1) * GCOLS])
        wvr = wv.reshape(NDC2, 2, P, GCOLS).transpose(2, 0, 1, 3)
        wv_by_g.append(_split8(np.ascontiguousarray(wvr)))
        bqk_by_g.append(bqk)
        bv_by_g.append(np.ascontiguousarray(
            b_qkv[2 * D + g * GCOLS: 2 * D + (g + 1) * GCOLS]).astype(
                np.float32))

    in_maps = []
    for c in range(NCORES):
        b, g = c // 2, c % 2
        xh, xl = xp_by_batch[b]
        wkqh, wkql = wkq_by_g[g]
        wvh, wvl = wv_by_g[g]
        in_maps.append({
            "xh": xh, "xl": xl,
            "wkqh": wkqh, "wkql": wkql,
            "wvh": wvh, "wvl": wvl,
            "wp": None,  # filled by caller (needs w_proj)
            "bqk": bqk_by_g[g], "bv": bv_by_g[g],
            "tri": tri, "eye": eye,
        })
    return in_maps


def full_in_maps(x, w_qkv, b_qkv, w_proj):
    w_proj = np.asarray(w_proj, np.float32) * WS
    in_maps = host_inputs(x, w_qkv, b_qkv)
    for c in range(NCORES):
        g = c % 2
        in_maps[c]["wp"] = np.ascontiguousarray(
            w_proj[g * GCOLS:(g + 1) * GCOLS, :]).astype(BF16)
    return in_maps


def gather(results, b_proj):
    out = np.zeros((B, T, D), np.float32)
    for c in range(NCORES):
        out[c // 2] += results[c]["outp"].astype(np.float32)
    out += np.asarray(b_proj, np.float32)[None, None, :]
    return out


_NC_CACHE = None


def kernel(x, w_qkv, b_qkv, w_proj, b_proj):
    global _NC_CACHE
    if _NC_CACHE is None:
        _NC_CACHE = build_nc()
    in_maps = full_in_maps(x, w_qkv, b_qkv, w_proj)
    res = run_bass_kernel_spmd(_NC_CACHE, in_maps, core_ids=list(range(NCORES)))
    return gather(res.results, b_proj)


if __name__ == "__main__":
    rng = np.random.default_rng(0)
    x = rng.standard_normal((B, T, D), dtype=np.float32)
    w_qkv = rng.standard_normal((D, 3 * D), dtype=np.float32) / np.sqrt(D)
    b_qkv = np.zeros(3 * D, np.float32)
    w_proj = rng.standard_normal((D, D), dtype=np.float32) / np.sqrt(D)
    b_proj = np.zeros(D, np.float32)
    out = kernel(x, w_qkv, b_qkv, w_proj, b_proj)
    print(out.shape, out.dtype)


# revision 59
# speedup vs baseline: 1.0791x; 1.0021x over previous
"""Causal self-attention Trainium2 kernel (v2).

Problem: B=4, T=2048, D=1024, H=16 heads (hd=64).
Sharding: 8 cores; core c -> batch c//2, heads (c%2)*8 .. +8.
Each core computes a partial output projection (its 512 rows of w_proj);
host sums the two partials per batch and adds b_proj.

v2 changes over the 247us baseline:
  - QKV projections run as fp8e4 DoubleRow 3-chain GEMMs
    (xh@wh + xh@wl + xl@wh, hi/lo splits prepared on host, w scaled by
    16 so the lo residual clears the fp8 denormal floor).  0.75x engine
    cycles AND 256-deep contraction per matmul.
  - P@V is flipped: out[q,65] = pts_block.T @ V_aug with N=65 columns
    per (q-block, k-block) instead of N=512 per k-block -- half the PE
    cycles.  The 8 per-group accumulators (2 heads x 4 q-subblocks)
    live in one [128,8,128]-padded PSUM tile (2 banks).  Denominators
    land per-partition, so normalization is a [128,2] reciprocal + one
    stride0-broadcast tensor_tensor -- no GpSimd partition_broadcast,
    no [1,512] reciprocals, no staging copies.
  - A is produced in natural [t, d] layout and PE-transposed (identity
    matmul, 53ns each) into A^T for the output projection lhsT.
  - Scale bookkeeping: Q,K,V carried at 16x (w scaled on host), scores
    at 256x (exp scale 0.125/256), P at 64x (exp bias ln 64, which also
    lifts softmax weights out of the bf16/fp8 denormal zone), A at 16x,
    out-proj product at 256x, divided back in the final PSUM->SBUF copy.

Schedule: same machinery as the baseline -- batched priority-ordered
input DMA ladder, deadline-ordered filler generators pumped between
attention steps, P@V emitted one ki late so the in-order PE queue never
waits on the Exp stream; normalize/transpose for q-subblock accumulators
emitted as soon as their last k-block lands (staggered across the ki
loop, transposes deferred one step).
"""

import math
import os
import sys
from collections import deque

PUMP_SAFE_N = int(os.environ.get("K_PUMP_SAFE", "3"))
PUMP_HI = int(os.environ.get("K_PUMP_HI", "2"))
PUMP_LO = int(os.environ.get("K_PUMP_LO", "1"))
PV_LATE = int(os.environ.get("K_PV_LATE", "2"))

for _p in ("/opt/trn_rl_repo",):
    if _p not in sys.path:
        sys.path.insert(0, _p)

import numpy as np
import ml_dtypes

import concourse.bass as bass
import concourse.mybir as mybir
import concourse.tile as tile
from concourse import bacc
from concourse.bass_utils import run_bass_kernel_spmd

BF16 = ml_dtypes.bfloat16
E4 = ml_dtypes.float8_e4m3fn

B, T, D = 4, 2048, 1024
H, HD = 16, 64
NCORES = 8
HPC = 8                  # heads per core
GCOLS = HPC * HD         # 512 columns of qkv per core per q/k/v
P = 128
NDC2 = 4                 # 4 DoubleRow contraction pairs of 256
NTT = T // P             # 16 t-tiles of 128
NQC = T // 512           # 4 q-chunks of 512
NMC = GCOLS // P         # 4 M-chunks per Q^T / K^T (2 heads each)

WS = 16.0                # host-side weight scale (w * 16)
EXP_SCALE = 0.125 / (WS * WS)      # scores carried at 256x
EXP_BIAS = math.log(64.0)          # P carried at 64x
OUT_SCALE = 1.0 / (WS * WS)        # A at 16x, wp at 16x -> /256

DR = mybir.MatmulPerfMode.DoubleRow


def _ap3(dram, off, part_stride, nparts, mid_stride, nmid, inner):
    """3-level DRAM access pattern: [partition, mid, contiguous-inner]."""
    a = dram.ap()
    return bass.AP(tensor=a.tensor, offset=a.offset + off,
                   ap=[[part_stride, nparts], [mid_stride, nmid], [1, inner]])


def build_nc(trace_sim: bool = False):
    f32 = mybir.dt.float32
    bf16 = mybir.dt.bfloat16
    fp8 = mybir.dt.float8e4

    nc = bacc.Bacc("TRN2", target_bir_lowering=False, debug=False,
                   num_devices=NCORES)

    # x^T hi/lo splits: [p, c, i, t] = split(x[t, 256c + 128i + p])
    xh_d = nc.dram_tensor("xh", [P, NDC2, 2, T], fp8, kind="ExternalInput")
    xl_d = nc.dram_tensor("xl", [P, NDC2, 2, T], fp8, kind="ExternalInput")
    # wkq hi/lo: [p, m, c, i, 256] -- per-m [K_m | Q_m] 128-col pairs,
    # rows regrouped into DoubleRow pairs, values scaled by 16
    wkqh_d = nc.dram_tensor("wkqh", [P, NMC, NDC2, 2, 256], fp8,
                            kind="ExternalInput")
    wkql_d = nc.dram_tensor("wkql", [P, NMC, NDC2, 2, 256], fp8,
                            kind="ExternalInput")
    wvh_d = nc.dram_tensor("wvh", [P, NDC2, 2, GCOLS], fp8,
                           kind="ExternalInput")
    wvl_d = nc.dram_tensor("wvl", [P, NDC2, 2, GCOLS], fp8,
                           kind="ExternalInput")
    wp_d = nc.dram_tensor("wp", [GCOLS, D], bf16, kind="ExternalInput")
    # bqk: col 2m = bias for K_m block, col 2m+1 = bias for Q_m block (16x)
    bqk_d = nc.dram_tensor("bqk", [P, 2 * NMC], f32, kind="ExternalInput")
    bv_d = nc.dram_tensor("bv", [GCOLS], f32, kind="ExternalInput")
    tri_d = nc.dram_tensor("tri", [P, P], bf16, kind="ExternalInput")
    eye_d = nc.dram_tensor("eye", [P, P], bf16, kind="ExternalInput")
    out_d = nc.dram_tensor("outp", [T, D], bf16, kind="ExternalOutput")

    with tile.TileContext(nc, trace_sim=trace_sim) as tc:
        with (
            tc.tile_pool(name="consts", bufs=1) as consts,
            tc.tile_pool(name="weights", bufs=1) as weights,
            tc.tile_pool(name="acts", bufs=1) as acts,
            tc.tile_pool(name="pt", bufs=6) as ptp,
            tc.tile_pool(name="norm", bufs=4) as normp,
            tc.tile_pool(name="outs", bufs=3) as outsp,
            tc.tile_pool(name="ps_mm", bufs=2, space="PSUM") as ps_mm,
            tc.tile_pool(name="ps_st", bufs=2, space="PSUM") as ps_st,
            tc.tile_pool(name="ps_po", bufs=1, space="PSUM") as ps_po,
            tc.tile_pool(name="ps_pb", bufs=1, space="PSUM") as ps_pb,
        ):
            xh_sb = acts.tile([P, NDC2, 2, T], fp8)
            xl_sb = acts.tile([P, NDC2, 2, T], fp8)
            wkqh_sb = weights.tile([P, NMC, NDC2, 2, 256], fp8)
            wkql_sb = weights.tile([P, NMC, NDC2, 2, 256], fp8)
            wvh_sb = weights.tile([P, NDC2, 2, GCOLS], fp8)
            wvl_sb = weights.tile([P, NDC2, 2, GCOLS], fp8)
            wp_sb = weights.tile([P, NMC, D], bf16)

            # ---- input DMA ladders: hi tensors on the SP HWDGE queue,
            # lo tensors + wp on the Activation HWDGE queue, both in
            # consumption-priority order ----
            def dma_x(eng, xd, xs, t0, t1):
                # slice [:, :, :, t0:t1]; (c, i) merge to one stride-T dim
                eng.dma_start(
                    xs[:, :, :, t0:t1],
                    bass.AP(tensor=xd.ap().tensor,
                            offset=xd.ap().offset + t0,
                            ap=[[NDC2 * 2 * T, P], [T, NDC2 * 2],
                                [1, t1 - t0]]))

            def dma_wkq(eng, wd, ws, m):
                # per-m slice, contiguous 2KB per partition on both sides
                eng.dma_start(
                    ws[:, m],
                    bass.AP(tensor=wd.ap().tensor,
                            offset=wd.ap().offset + m * 2048,
                            ap=[[NMC * 2048, P], [1, 2048]]))

            # Four parallel queues. Pool SWDGE: small consts first (the
            # V bias-add needs bv_rep, the first diagonal mask needs tri),
            # then wv hi/lo + wp; SP: xh + wkqh; ACT: xl + wkql.
            # Pool SWDGE, ordered by first use: wv hi/lo (V chains), then
            # the consts (bias-add, masks, transposes), then wp (late).
            nc.gpsimd.dma_start(wvh_sb[:], wvh_d.ap())
            bv_rep = consts.tile([P, GCOLS], f32)
            bv_ap = bv_d.ap()
            nc.gpsimd.dma_start(
                bv_rep[:],
                bass.AP(tensor=bv_ap.tensor, offset=bv_ap.offset,
                        ap=[[0, P]] + list(bv_ap.ap)),
            )
            bqk_sb = consts.tile([P, 2 * NMC], f32)
            nc.gpsimd.dma_start(bqk_sb[:], bqk_d.ap())
            tri_sb = consts.tile([P, P], bf16)
            nc.gpsimd.dma_start(tri_sb[:], tri_d.ap())
            eye_sb = consts.tile([P, P], bf16)
            nc.gpsimd.dma_start(eye_sb[:], eye_d.ap())
            nc.gpsimd.dma_start(wp_sb[:, :, :],
                                _ap3(wp_d, 0, D, P, P * D, NMC, D))

            # SP / ACT HWDGE: x slices early (the DR projections burn
            # through bytes fast); wkq m1-3 only feed (m>=1, *) groups.
            dma_x(nc.sync, xh_d, xh_sb, 0, 512)
            dma_wkq(nc.sync, wkqh_d, wkqh_sb, 0)
            dma_x(nc.sync, xh_d, xh_sb, 512, 1024)
            dma_x(nc.sync, xh_d, xh_sb, 1024, 2048)
            for m in range(1, NMC):
                dma_wkq(nc.sync, wkqh_d, wkqh_sb, m)

            nc.scalar.dma_start(wvl_sb[:], wvl_d.ap())
            dma_x(nc.scalar, xl_d, xl_sb, 0, 512)
            dma_wkq(nc.scalar, wkql_d, wkql_sb, 0)
            dma_x(nc.scalar, xl_d, xl_sb, 512, 1024)
            dma_x(nc.scalar, xl_d, xl_sb, 1024, 2048)
            for m in range(1, NMC):
                dma_wkq(nc.scalar, wkql_d, wkql_sb, m)

            # per-partition exp bias (ln 64 -> P carried at 64x)
            ebias = consts.tile([P, 1], f32)
            nc.vector.memset(ebias[:], EXP_BIAS)

            # zeros for the K=1 PSUM-bank-clearing matmuls (start=True
            # zeroes a whole 2KB bank, so the 8 P@V accumulators sharing
            # two banks get one explicit zeroing matmul per bank instead;
            # fp8 DoubleRow so each costs 256 instead of 512 PE cycles)
            zz = consts.tile([1, 2, 640], fp8)
            nc.vector.memset(zz[:], 0.0)

            # warm the ScalarE Exp table during the startup DMA window
            warm = consts.tile([1, 1], f32)
            nc.vector.memset(warm[:], 0.0)
            nc.scalar.activation(warm[:], warm[:],
                                 mybir.ActivationFunctionType.Exp)

            # V natural + ones column: [128, tt, head, 65] (values at 16x)
            V_sb = acts.tile([P, NTT, HPC, HD + 1], bf16)
            nc.vector.memset(V_sb[:, :, :, HD], 1.0)

            QT_sb = acts.tile([P, NMC, T], bf16)
            KT_sb = acts.tile([P, NMC, T], bf16)
            AT_sb = acts.tile([P, NMC, T], bf16)

            # ---------------- filler generators ----------------
            def dr_chains(lhs_pair, rhs_pair):
                """(lhsT, rhs) fp8 tensors -> 3-chain DoubleRow schedule:
                yields (lhs_sb, rhs_sb, c, is_first, is_last)."""
                (lh, ll), (rh, rl) = lhs_pair, rhs_pair
                chains = [(lh, rh), (lh, rl), (ll, rh)]
                n = len(chains) * NDC2
                i = 0
                for ls, rs in chains:
                    for c in range(NDC2):
                        yield ls, rs, c, i == 0, i == n - 1
                        i += 1

            def gen_v(tt):
                pv = ps_mm.tile([P, 512], f32, tag="mm", name=f"pv{tt}")
                sched = list(dr_chains((xh_sb, xl_sb), (wvh_sb, wvl_sb)))
                for ls, rs, c, first, last in sched:
                    nc.tensor.matmul(
                        pv[:],
                        ls[:, c, :, tt * P:(tt + 1) * P],
                        rs[:, c, :, :],
                        start=first, stop=last, perf_mode=DR,
                    )
                    if not last:
                        yield
                nc.vector.tensor_tensor(
                    V_sb[:, tt, :, 0:HD],
                    pv[:].rearrange("p (h d) -> p h d", h=HPC),
                    bv_rep[:].rearrange("p (h d) -> p h d", h=HPC),
                    mybir.AluOpType.add,
                )

            def gen_kq(is_q, m, tc4):
                off = P if is_q else 0
                pq = ps_mm.tile([P, 512], f32, tag="mm",
                                name=f"p{'q' if is_q else 'k'}{m}_{tc4}")
                sched = list(dr_chains((wkqh_sb, wkql_sb), (xh_sb, xl_sb)))
                for ls, rs, c, first, last in sched:
                    nc.tensor.matmul(
                        pq[:],
                        ls[:, m, c, :, off:off + P],
                        rs[:, c, :, tc4 * 512:(tc4 + 1) * 512],
                        start=first, stop=last, perf_mode=DR,
                    )
                    if not last:
                        yield
                dst = QT_sb if is_q else KT_sb
                bcol = 2 * m + (1 if is_q else 0)
                nc.vector.tensor_scalar_add(
                    dst[:, m, tc4 * 512:(tc4 + 1) * 512],
                    pq[:], bqk_sb[:, bcol:bcol + 1],
                )

            at_ready = set()

            def gen_out(tt, ncol, tail=False, deng=None, eager=False):
                pp = ps_mm.tile([P, 512], f32, tag="mm",
                                name=f"pp{tt}_{ncol}")
                for hc in range(NMC):
                    if eager and hc == NMC - 1:
                        # final chunk reads AT[:, 3, tt]: spin until that
                        # A^T block's copy has been EMITTED (program order
                        # is the only ordering the tile deps can see)
                        while tt not in at_ready:
                            yield
                    nc.tensor.matmul(
                        pp[:],
                        AT_sb[:, hc, tt * P:(tt + 1) * P],
                        wp_sb[:, hc, ncol * 512:(ncol + 1) * 512],
                        start=(hc == 0), stop=(hc == NMC - 1),
                    )
                    if hc < NMC - 1:
                        yield
                ot = outsp.tile([P, 512], bf16, tag="ot", bufs=7)
                if tail:
                    nc.scalar.activation(ot[:], pp[:],
                                         mybir.ActivationFunctionType.Copy,
                                         scale=OUT_SCALE)
                else:
                    nc.vector.tensor_scalar(ot[:], pp[:], OUT_SCALE, None,
                                            mybir.AluOpType.mult)
                (deng or nc.sync).dma_start(
                    out_d[tt * P:(tt + 1) * P,
                          ncol * 512:(ncol + 1) * 512],
                    ot[:],
                )

            fillers = deque()   # [deadline, generator, started]

            def pump(n):
                while n > 0 and fillers:
                    ent = fillers[0]
                    try:
                        next(ent[1])
                        ent[2] = True
                    except StopIteration:
                        fillers.popleft()
                    n -= 1

            def pump_safe(n):
                """Advance only already-started chains (their PSUM slot is
                held; further matmuls have no new waits) -- safe to emit
                ahead of the scores matmuls without risking a new stall."""
                while n > 0 and fillers and fillers[0][2]:
                    try:
                        next(fillers[0][1])
                    except StopIteration:
                        fillers.popleft()
                    n -= 1

            def drain_until(deadline):
                while fillers and fillers[0][0] <= deadline:
                    for _ in fillers.popleft()[1]:
                        pass

            def run_gen(g):
                for _ in g:
                    pass

            # ---------------- startup compute ----------------
            for tt in range(4):
                run_gen(gen_v(tt))
            run_gen(gen_kq(False, 0, 0))
            run_gen(gen_kq(True, 0, 0))

            # deadline-ordered filler queue. Deadline = one group BEFORE
            # the (mch, qc) at whose start the output is first consumed,
            # so a slow pump can't leave the consuming group's first
            # scores matmul waiting on a just-drained projection chain.
            def prev_group(m, qc):
                # shift deadlines one group early so the consuming group's
                # first scores never wait on a just-drained chain -- but
                # not for m==0 (startup DMA still in flight there)
                if m == 0:
                    return (m, qc)
                return (m, qc - 1) if qc > 0 else (m - 1, NQC - 1)

            for m in range(NMC):
                for qc in range(NQC):
                    if m == 0 and qc == 0:
                        continue
                    dl = prev_group(m, qc)
                    fillers.append([dl, gen_kq(False, m, qc), False])
                    fillers.append([dl, gen_kq(True, m, qc), False])
                    if m == 0:
                        for tt in range(4 * qc, 4 * qc + 4):
                            fillers.append([dl, gen_v(tt), False])

            # ---------------- attention main loop ----------------
            def norm_qsub(pot, qsub):
                """Normalize accumulators (j=0,1) for q-subblock qsub into
                A natural layout; return the A_nat tile for transposition.
                pot slot order: 2*(qsub%2) + j."""
                s0 = 2 * (qsub % 2)
                rs = normp.tile([P, 2], f32, tag="rs")
                nc.vector.reciprocal(
                    rs[:], pot[:, s0:s0 + 2, HD:HD + 1])
                an = normp.tile([P, 2, HD], bf16, tag="an")
                rs_ap = rs[:]
                rs_b = bass.AP(
                    tensor=rs_ap.tensor, offset=rs_ap.offset,
                    ap=[list(rs_ap.ap[0]), [1, 2], [0, HD]])
                nc.vector.tensor_tensor(
                    an[:], pot[:, s0:s0 + 2, 0:HD], rs_b,
                    mybir.AluOpType.mult)
                return an

            def transpose_qsub(an, mch, qc, qsub):
                tr = ps_mm.tile([P, P], bf16, tag="mm", name="tr")
                nc.tensor.transpose(tr[:], an[:], eye_sb[:])
                nc.vector.tensor_copy(
                    AT_sb[:, mch, qc * 512 + qsub * P:
                          qc * 512 + (qsub + 1) * P], tr[:])
                if mch == NMC - 1:
                    at_ready.add(4 * qc + qsub)

            for mch in range(NMC):
                for qc in range(NQC):
                    drain_until((mch, qc))
                    # separate 1-bank tiles (separate pools): the next
                    # group's qsub0/1 tile only WARs against qsub0/1
                    # normalize reads, which happen early in this group
                    poA = ps_po.tile([P, 4, P], f32, tag="poA",
                                     name=f"poA{mch}_{qc}")
                    poB = ps_pb.tile([P, 4, P], f32, tag="poB",
                                     name=f"poB{mch}_{qc}")
                    po2 = [poA, poB]

                    def zero_po(half):
                        # one start=True matmul per 2KB accumulator bank
                        # (start zeroes a whole bank), emitted just before
                        # the bank's first P@V
                        nc.tensor.matmul(
                            po2[half][:, :, :],
                            zz[0:1, :, 0:P], zz[0:1, :, P:P + 512],
                            start=True, stop=True,
                            skip_group_check=True, perf_mode=DR,
                        )
                    if (mch, qc) == (NMC - 1, NQC - 1):
                        for s_ in range(4):
                            for ncol in range(2):
                                fillers.append(
                                    [(8, qc, s_),
                                     gen_out(12 + s_, ncol,
                                             tail=(ncol == 1),
                                             deng=(nc.scalar if ncol
                                                   else nc.sync),
                                             eager=True), False])
                    nki = 4 * qc + 4
                    pv_pending = deque()
                    pending_tr = []
                    deferred = []
                    cur_ki = [0]
                    zeroed = [False, False]

                    def pop_pv():
                        ki_, pts_ = pv_pending.popleft()
                        for half in range(2):
                            if not zeroed[half]:
                                zero_po(half)
                                zeroed[half] = True
                        do_pv(ki_, pts_)

                    def do_pv(ki, pts):
                        # P@V for k-block ki into accumulators (two-late)
                        for qsub in range(4):
                            qlim = 4 * qc + qsub
                            if ki > qlim:
                                continue
                            for j in range(2):
                                nc.tensor.matmul(
                                    po2[qsub // 2][:, 2 * (qsub % 2) + j,
                                                   0:HD + 1],
                                    pts[:, j, qsub * P:(qsub + 1) * P],
                                    V_sb[:, ki, 2 * mch + j, :],
                                    start=False, stop=(ki == qlim),
                                    skip_group_check=True,
                                )
                        if ki >= 4 * qc:
                            qsub = ki - 4 * qc
                            an = norm_qsub(po2[qsub // 2], qsub)
                            pending_tr.append((cur_ki[0], an, mch, qc, qsub))

                    for ki in range(nki):
                        off = max(0, ki - 4 * qc) * P
                        while deferred and deferred[0][0] <= ki:
                            fillers.append(deferred.pop(0)[1])
                        # continue in-flight filler chains ahead of the
                        # scores matmuls (no new waits -> absorbs ACT lag
                        # without risking a fresh-chain stall)
                        pump_safe(PUMP_SAFE_N)
                        pshat = ps_st.tile([P, 2, 512], f32, tag="st")
                        pts = ptp.tile([P, 2, 512], bf16, tag="pt")
                        for j in range(2):
                            part = j * 64
                            nc.tensor.matmul(
                                pshat[:, j, off:512],
                                KT_sb[part:part + 64, mch,
                                      ki * P:(ki + 1) * P],
                                QT_sb[part:part + 64, mch,
                                      qc * 512 + off:(qc + 1) * 512],
                                start=True, stop=True,
                            )
                        nc.scalar.activation(
                            pts[:, :, off:512], pshat[:, :, off:512],
                            mybir.ActivationFunctionType.Exp,
                            scale=EXP_SCALE, bias=ebias[:],
                        )
                        if ki >= 4 * qc:
                            # diagonal block: zero out q < k entries.
                            # On GpSimd (SBUF-only engine, otherwise idle)
                            # so the exp->mask chain never backs up the
                            # DVE queue; tri broadcast over j via a
                            # 0-stride AP dim.
                            tri_ap = tri_sb[:]
                            tri_b = bass.AP(
                                tensor=tri_ap.tensor, offset=tri_ap.offset,
                                ap=[list(tri_ap.ap[0]), [0, 2], [1, P]])
                            nc.vector.tensor_tensor(
                                pts[:, :, off:off + P],
                                pts[:, :, off:off + P],
                                tri_b, mybir.AluOpType.mult,
                            )
                        cur_ki[0] = ki
                        pump(8 if (mch, qc) == (NMC - 1, NQC - 1) else (PUMP_HI if mch == NMC - 1 else PUMP_LO))
                        # flush transposes whose normalize is >= 2 steps old
                        while pending_tr and pending_tr[0][0] <= ki - (1 if (mch, qc) == (NMC - 1, NQC - 1) else 3):
                            _, an_, m_, q_, s_ = pending_tr.pop(0)
                            transpose_qsub(an_, m_, q_, s_)
                            if mch == NMC - 1 and qc < NQC - 1:
                                # mch 3: this row-block's A^T is complete;
                                # its out-proj becomes pump fodder after a
                                # 2-iteration grace for the DVE copy
                                for ncol in range(2):
                                    deferred.append(
                                        (cur_ki[0] + 2,
                                         [(8, q_, s_),
                                          gen_out(4 * q_ + s_, ncol),
                                          False]))
                        pv_pending.append((ki, pts))
                        if len(pv_pending) > (1 if (mch, qc) == (NMC - 1, NQC - 1) else PV_LATE):
                            pop_pv()
                    # remaining k-blocks (two-late, post loop)
                    while pv_pending:
                        pop_pv()
                    while deferred:
                        fillers.append(deferred.pop(0)[1])
                    while pending_tr:
                        _, an_, m_, q_, s_ = pending_tr.pop(0)
                        transpose_qsub(an_, m_, q_, s_)
                        if mch == NMC - 1 and qc < NQC - 1:
                            for ncol in range(2):
                                fillers.append(
                                    [(8, q_, s_),
                                     gen_out(4 * q_ + s_, ncol), False])
            # ---- tail: drain remaining fillers round-robin ----
            wave = deque(e[1] for e in fillers)
            while wave:
                g = wave.popleft()
                try:
                    next(g)
                except StopIteration:
                    continue
                wave.append(g)

    nc.compile()
    return nc


def _split8(a):
    """fp32 -> (hi, lo) e4m3 split."""
    hi = a.astype(E4)
    lo = (a - hi.astype(np.float32)).astype(E4)
    return hi, lo


def host_inputs(x, w_qkv, b_qkv):
    """Per-core input maps. Core c -> batch c//2, head group c%2."""
    x = np.asarray(x, np.float32)
    w_qkv = np.asarray(w_qkv, np.float32) * WS
    b_qkv = np.asarray(b_qkv, np.float32) * WS
    tri = (np.arange(P)[None, :] >= np.arange(P)[:, None]).astype(BF16)
    eye = np.eye(P, dtype=np.float32).astype(BF16)

    # x^T regrouped to DoubleRow pairs: [p, c, i, t] = x[t, 256c+128i+p]
    xp_by_batch = []
    for b in range(B):
        xT = np.ascontiguousarray(x[b].T)          # [D, T]
        xr = xT.reshape(NDC2, 2, P, T).transpose(2, 0, 1, 3)  # [p,c,i,t]
        xp_by_batch.append(_split8(np.ascontiguousarray(xr)))

    wkq_by_g = []
    wv_by_g = []
    bqk_by_g = []
    bv_by_g = []
    for g in range(2):
        # interleaved [K_m | Q_m] 128-col pairs
        wkq = np.empty((D, 2 * GCOLS), np.float32)
        bqk = np.empty((P, 2 * NMC), np.float32)
        for m in range(NMC):
            qs = g * GCOLS + m * P
            ks = D + g * GCOLS + m * P
            wkq[:, 256 * m:256 * m + P] = w_qkv[:, ks:ks + P]
            wkq[:, 256 * m + P:256 * (m + 1)] = w_qkv[:, qs:qs + P]
            bqk[:, 2 * m] = b_qkv[ks:ks + P]
            bqk[:, 2 * m + 1] = b_qkv[qs:qs + P]
        # [d, col] -> [p, m, c, i, 256]: d = 256c+128i+p, col = 256m+w
        wkqr = wkq.reshape(NDC2, 2, P, NMC, 256).transpose(2, 3, 0, 1, 4)
        wkq_by_g.append(_split8(np.ascontiguousarray(wkqr)))
        wv = np.ascontiguousarray(
            w_qkv[:, 2 * D + g * GCOLS: 2 * D + (g + 1) * GCOLS])
        wvr = wv.reshape(NDC2, 2, P, GCOLS).transpose(2, 0, 1, 3)
        wv_by_g.append(_split8(np.ascontiguousarray(wvr)))
        bqk_by_g.append(bqk)
        bv_by_g.append(np.ascontiguousarray(
            b_qkv[2 * D + g * GCOLS: 2 * D + (g + 1) * GCOLS]).astype(
                np.float32))

    in_maps = []
    for c in range(NCORES):
        b, g = c // 2, c % 2
        xh, xl = xp_by_batch[b]
        wkqh, wkql = wkq_by_g[g]
        wvh, wvl = wv_by_g[g]
        in_maps.append({
            "xh": xh, "xl": xl,
            "wkqh": wkqh, "wkql": wkql,
            "wvh": wvh, "wvl": wvl,
            "wp": None,  # filled by caller (needs w_proj)
            "bqk": bqk_by_g[g], "bv": bv_by_g[g],
            "tri": tri, "eye": eye,
        })
    return in_maps


def full_in_maps(x, w_qkv, b_qkv, w_proj):
    w_proj = np.asarray(w_proj, np.float32) * WS
    in_maps = host_inputs(x, w_qkv, b_qkv)
    for c in range(NCORES):
        g = c % 2
        in_maps[c]["wp"] = np.ascontiguousarray(
            w_proj[g * GCOLS:(g + 1) * GCOLS, :]).astype(BF16)
    return in_maps


def gather(results, b_proj):
    out = np.zeros((B, T, D), np.float32)
    for c in range(NCORES):
        out[c // 2] += results[c]["outp"].astype(np.float32)
    out += np.asarray(b_proj, np.float32)[None, None, :]
    return out


_NC_CACHE = None


def kernel(x, w_qkv, b_qkv, w_proj, b_proj):
    global _NC_CACHE
    if _NC_CACHE is None:
        _NC_CACHE = build_nc()
    in_maps = full_in_maps(x, w_qkv, b_qkv, w_proj)
    res = run_bass_kernel_spmd(_NC_CACHE, in_maps, core_ids=list(range(NCORES)))
    return gather(res.results, b_proj)


if __name__ == "__main__":
    rng = np.random.default_rng(0)
    x = rng.standard_normal((B, T, D), dtype=np.float32)
    w_qkv = rng.standard_normal((D, 3 * D), dtype=np.float32) / np.sqrt(D)
    b_qkv = np.zeros(3 * D, np.float32)
    w_proj = rng.standard_normal((D, D), dtype=np.float32) / np.sqrt(D)
    b_proj = np.zeros(D, np.float32)
    out = kernel(x, w_qkv, b_qkv, w_proj, b_proj)
    print(out.shape, out.dtype)


# revision 70
# speedup vs baseline: 1.0798x; 1.0006x over previous
"""Causal self-attention Trainium2 kernel (v2).

Problem: B=4, T=2048, D=1024, H=16 heads (hd=64).
Sharding: 8 cores; core c -> batch c//2, heads (c%2)*8 .. +8.
Each core computes a partial output projection (its 512 rows of w_proj);
host sums the two partials per batch and adds b_proj.

v2 changes over the 247us baseline (-> 228.8us, rel err 4.8e-3):
  - QKV projections run as fp8e4 DoubleRow 3-chain GEMMs
    (xh@wh + xh@wl + xl@wh, hi/lo splits prepared on host, w scaled by
    16 so the lo residual clears the fp8 denormal floor).  0.75x engine
    cycles AND 256-deep contraction per matmul.  Scores / P@V / out-proj
    stay bf16: single-fp8 anywhere there costs ~2-4e-2 output error
    (measured), over the budget.
  - P@V is flipped: out[q,65] = pts_block.T @ V_aug with N=65 output
    columns per (q-block, k-block) instead of N=512 per k-block -- half
    the PE cycles.  The 8 per-group accumulators (2 heads x 4 q-sub-
    blocks) live in two 1-bank PSUM tiles from separate pools (qsub01 /
    qsub23), so the next group's first bank only WARs against normalize
    reads that happen early in the current group.  start=True zeroes a
    whole 2KB bank, so each bank gets one explicit K=1 zeroing matmul
    and the accumulators run start=False.
  - Denominators land per-partition (ones column of V_aug), so
    normalization is a [128,2] reciprocal + one stride0-broadcast
    tensor_tensor -- no partition_broadcast, no [1,512] reciprocals,
    no staging copies.  A is produced in natural [t,d] layout and
    PE-transposed (identity matmul, 53ns) into A^T for the out-proj.
  - Scale bookkeeping: Q,K,V carried at 16x (w scaled on host), scores
    at 256x (exp scale 0.125/256), P at 64x (exp bias ln 64, which also
    lifts softmax weights out of the bf16/fp8 denormal zone), A at 16x,
    out-proj product at 256x, divided back in the final PSUM->SBUF copy.

Schedule: 4 parallel input-DMA queues (SP/ACT HWDGE + Pool SWDGE) in
consumption-priority order; deadline-ordered filler generators (DR
projection chains, out-proj chains) pumped between attention steps,
with a pump_safe pass that only continues in-flight chains ahead of the
scores matmuls; P@V emitted two ki late so the in-order PE queue never
waits on the Exp stream; diagonal masks on the DVE with a j-broadcast
tri; per-q-subblock normalize emitted the moment its last k-block
lands, transposes a few steps later; the last group's out-projection
chains are emitted eagerly (hc 0-2 of A^T are long since ready) with a
spin-gate before the final hc chunk.
"""

import math
import os
import sys
from collections import deque

PUMP_SAFE_N = int(os.environ.get("K_PUMP_SAFE", "3"))
PUMP_HI = int(os.environ.get("K_PUMP_HI", "2"))
PUMP_LO = int(os.environ.get("K_PUMP_LO", "1"))
PV_LATE = int(os.environ.get("K_PV_LATE", "2"))

for _p in ("/opt/trn_rl_repo",):
    if _p not in sys.path:
        sys.path.insert(0, _p)

import numpy as np
import ml_dtypes

import concourse.bass as bass
import concourse.mybir as mybir
import concourse.tile as tile
from concourse import bacc
from concourse.bass_utils import run_bass_kernel_spmd

BF16 = ml_dtypes.bfloat16
E4 = ml_dtypes.float8_e4m3fn

B, T, D = 4, 2048, 1024
H, HD = 16, 64
NCORES = 8
HPC = 8                  # heads per core
GCOLS = HPC * HD         # 512 columns of qkv per core per q/k/v
P = 128
NDC2 = 4                 # 4 DoubleRow contraction pairs of 256
NTT = T // P             # 16 t-tiles of 128
NQC = T // 512           # 4 q-chunks of 512
NMC = GCOLS // P         # 4 M-chunks per Q^T / K^T (2 heads each)

WS = 16.0                # host-side weight scale (w * 16)
EXP_SCALE = 0.125 / (WS * WS)      # scores carried at 256x
EXP_BIAS = math.log(64.0)          # P carried at 64x
OUT_SCALE = 1.0 / (WS * WS)        # A at 16x, wp at 16x -> /256

DR = mybir.MatmulPerfMode.DoubleRow


def _ap3(dram, off, part_stride, nparts, mid_stride, nmid, inner):
    """3-level DRAM access pattern: [partition, mid, contiguous-inner]."""
    a = dram.ap()
    return bass.AP(tensor=a.tensor, offset=a.offset + off,
                   ap=[[part_stride, nparts], [mid_stride, nmid], [1, inner]])


def build_nc(trace_sim: bool = False):
    f32 = mybir.dt.float32
    bf16 = mybir.dt.bfloat16
    fp8 = mybir.dt.float8e4

    nc = bacc.Bacc("TRN2", target_bir_lowering=False, debug=False,
                   num_devices=NCORES)

    # x^T hi/lo splits: [p, c, i, t] = split(x[t, 256c + 128i + p])
    xh_d = nc.dram_tensor("xh", [P, NDC2, 2, T], fp8, kind="ExternalInput")
    xl_d = nc.dram_tensor("xl", [P, NDC2, 2, T], fp8, kind="ExternalInput")
    # wkq hi/lo: [p, m, c, i, 256] -- per-m [K_m | Q_m] 128-col pairs,
    # rows regrouped into DoubleRow pairs, values scaled by 16
    wkqh_d = nc.dram_tensor("wkqh", [P, NMC, NDC2, 2, 256], fp8,
                            kind="ExternalInput")
    wkql_d = nc.dram_tensor("wkql", [P, NMC, NDC2, 2, 256], fp8,
                            kind="ExternalInput")
    wvh_d = nc.dram_tensor("wvh", [P, NDC2, 2, GCOLS], fp8,
                           kind="ExternalInput")
    wvl_d = nc.dram_tensor("wvl", [P, NDC2, 2, GCOLS], fp8,
                           kind="ExternalInput")
    wp_d = nc.dram_tensor("wp", [GCOLS, D], bf16, kind="ExternalInput")
    # bqk: col 2m = bias for K_m block, col 2m+1 = bias for Q_m block (16x)
    bqk_d = nc.dram_tensor("bqk", [P, 2 * NMC], f32, kind="ExternalInput")
    bv_d = nc.dram_tensor("bv", [GCOLS], f32, kind="ExternalInput")
    tri_d = nc.dram_tensor("tri", [P, P], bf16, kind="ExternalInput")
    eye_d = nc.dram_tensor("eye", [P, P], bf16, kind="ExternalInput")
    out_d = nc.dram_tensor("outp", [T, D], bf16, kind="ExternalOutput")

    with tile.TileContext(nc, trace_sim=trace_sim) as tc:
        with (
            tc.tile_pool(name="consts", bufs=1) as consts,
            tc.tile_pool(name="weights", bufs=1) as weights,
            tc.tile_pool(name="acts", bufs=1) as acts,
            tc.tile_pool(name="pt", bufs=6) as ptp,
            tc.tile_pool(name="norm", bufs=4) as normp,
            tc.tile_pool(name="outs", bufs=3) as outsp,
            tc.tile_pool(name="ps_mm", bufs=2, space="PSUM") as ps_mm,
            tc.tile_pool(name="ps_st", bufs=2, space="PSUM") as ps_st,
            tc.tile_pool(name="ps_po", bufs=1, space="PSUM") as ps_po,
            tc.tile_pool(name="ps_pb", bufs=1, space="PSUM") as ps_pb,
        ):
            xh_sb = acts.tile([P, NDC2, 2, T], fp8)
            xl_sb = acts.tile([P, NDC2, 2, T], fp8)
            wkqh_sb = weights.tile([P, NMC, NDC2, 2, 256], fp8)
            wkql_sb = weights.tile([P, NMC, NDC2, 2, 256], fp8)
            wvh_sb = weights.tile([P, NDC2, 2, GCOLS], fp8)
            wvl_sb = weights.tile([P, NDC2, 2, GCOLS], fp8)
            wp_sb = weights.tile([P, NMC, D], bf16)

            # ---- input DMA ladders: hi tensors on the SP HWDGE queue,
            # lo tensors + wp on the Activation HWDGE queue, both in
            # consumption-priority order ----
            def dma_x(eng, xd, xs, t0, t1):
                # slice [:, :, :, t0:t1]; (c, i) merge to one stride-T dim
                eng.dma_start(
                    xs[:, :, :, t0:t1],
                    bass.AP(tensor=xd.ap().tensor,
                            offset=xd.ap().offset + t0,
                            ap=[[NDC2 * 2 * T, P], [T, NDC2 * 2],
                                [1, t1 - t0]]))

            def dma_wkq(eng, wd, ws, m):
                # per-m slice, contiguous 2KB per partition on both sides
                eng.dma_start(
                    ws[:, m],
                    bass.AP(tensor=wd.ap().tensor,
                            offset=wd.ap().offset + m * 2048,
                            ap=[[NMC * 2048, P], [1, 2048]]))

            # NOTE: the cost model serializes all DMA transfers on one
            # shared resource -- only the global order matters.  Priority:
            # V(0) chain deps (wvh, xh0, wvl, xl0), first-group consts
            # (bv_rep, tri, eye, bqk) and wkq m0, then the bulk x tail,
            # then late-needed weights (wkq m1-3, wp).
            nc.gpsimd.dma_start(wvh_sb[:], wvh_d.ap())
            dma_x(nc.sync, xh_d, xh_sb, 0, 512)
            nc.scalar.dma_start(wvl_sb[:], wvl_d.ap())
            dma_x(nc.sync, xl_d, xl_sb, 0, 512)
            bv_rep = consts.tile([P, GCOLS], f32)
            bv_ap = bv_d.ap()
            nc.gpsimd.dma_start(
                bv_rep[:],
                bass.AP(tensor=bv_ap.tensor, offset=bv_ap.offset,
                        ap=[[0, P]] + list(bv_ap.ap)),
            )
            dma_wkq(nc.scalar, wkqh_d, wkqh_sb, 0)
            dma_wkq(nc.scalar, wkql_d, wkql_sb, 0)
            tri_sb = consts.tile([P, P], bf16)
            nc.gpsimd.dma_start(tri_sb[:], tri_d.ap())
            bqk_sb = consts.tile([P, 2 * NMC], f32)
            nc.gpsimd.dma_start(bqk_sb[:], bqk_d.ap())
            eye_sb = consts.tile([P, P], bf16)
            nc.gpsimd.dma_start(eye_sb[:], eye_d.ap())
            dma_x(nc.sync, xh_d, xh_sb, 512, 1024)
            dma_x(nc.scalar, xl_d, xl_sb, 512, 1024)
            dma_x(nc.sync, xh_d, xh_sb, 1024, 2048)
            dma_x(nc.scalar, xl_d, xl_sb, 1024, 2048)
            for m in range(1, NMC):
                dma_wkq(nc.sync, wkqh_d, wkqh_sb, m)
            for m in range(1, NMC):
                dma_wkq(nc.scalar, wkql_d, wkql_sb, m)
            nc.gpsimd.dma_start(wp_sb[:, :, :],
                                _ap3(wp_d, 0, D, P, P * D, NMC, D))

            # per-partition exp bias (ln 64 -> P carried at 64x)
            ebias = consts.tile([P, 1], f32)
            nc.vector.memset(ebias[:], EXP_BIAS)

            # zeros for the K=1 PSUM-bank-clearing matmuls (start=True
            # zeroes a whole 2KB bank, so the 8 P@V accumulators sharing
            # two banks get one explicit zeroing matmul per bank instead;
            # fp8 DoubleRow so each costs 256 instead of 512 PE cycles)
            zz = consts.tile([1, 2, 640], fp8)
            nc.vector.memset(zz[:], 0.0)

            # warm the ScalarE Exp table during the startup DMA window
            warm = consts.tile([1, 1], f32)
            nc.vector.memset(warm[:], 0.0)
            nc.scalar.activation(warm[:], warm[:],
                                 mybir.ActivationFunctionType.Exp)

            # V natural + ones column: [128, tt, head, 65] (values at 16x)
            V_sb = acts.tile([P, NTT, HPC, HD + 1], bf16)
            nc.vector.memset(V_sb[:, :, :, HD], 1.0)

            QT_sb = acts.tile([P, NMC, T], bf16)
            KT_sb = acts.tile([P, NMC, T], bf16)
            AT_sb = acts.tile([P, NMC, T], bf16)

            # ---------------- filler generators ----------------
            def dr_chains(lhs_pair, rhs_pair):
                """(lhsT, rhs) fp8 tensors -> 3-chain DoubleRow schedule:
                yields (lhs_sb, rhs_sb, c, is_first, is_last)."""
                (lh, ll), (rh, rl) = lhs_pair, rhs_pair
                chains = [(lh, rh), (lh, rl), (ll, rh)]
                n = len(chains) * NDC2
                i = 0
                for ls, rs in chains:
                    for c in range(NDC2):
                        yield ls, rs, c, i == 0, i == n - 1
                        i += 1

            def gen_v(tt):
                pv = ps_mm.tile([P, 512], f32, tag="mm", name=f"pv{tt}")
                sched = list(dr_chains((xh_sb, xl_sb), (wvh_sb, wvl_sb)))
                for ls, rs, c, first, last in sched:
                    nc.tensor.matmul(
                        pv[:],
                        ls[:, c, :, tt * P:(tt + 1) * P],
                        rs[:, c, :, :],
                        start=first, stop=last, perf_mode=DR,
                    )
                    if not last:
                        yield
                nc.vector.tensor_tensor(
                    V_sb[:, tt, :, 0:HD],
                    pv[:].rearrange("p (h d) -> p h d", h=HPC),
                    bv_rep[:].rearrange("p (h d) -> p h d", h=HPC),
                    mybir.AluOpType.add,
                )

            def gen_kq(is_q, m, tc4):
                off = P if is_q else 0
                pq = ps_mm.tile([P, 512], f32, tag="mm",
                                name=f"p{'q' if is_q else 'k'}{m}_{tc4}")
                sched = list(dr_chains((wkqh_sb, wkql_sb), (xh_sb, xl_sb)))
                for ls, rs, c, first, last in sched:
                    nc.tensor.matmul(
                        pq[:],
                        ls[:, m, c, :, off:off + P],
                        rs[:, c, :, tc4 * 512:(tc4 + 1) * 512],
                        start=first, stop=last, perf_mode=DR,
                    )
                    if not last:
                        yield
                dst = QT_sb if is_q else KT_sb
                bcol = 2 * m + (1 if is_q else 0)
                nc.vector.tensor_scalar_add(
                    dst[:, m, tc4 * 512:(tc4 + 1) * 512],
                    pq[:], bqk_sb[:, bcol:bcol + 1],
                )

            at_ready = set()

            def gen_out(tt, ncol, tail=False, deng=None, eager=False):
                pp = ps_mm.tile([P, 512], f32, tag="mm",
                                name=f"pp{tt}_{ncol}")
                for hc in range(NMC):
                    if eager and hc == NMC - 1:
                        # final chunk reads AT[:, 3, tt]: spin until that
                        # A^T block's copy has been EMITTED (program order
                        # is the only ordering the tile deps can see)
                        while tt not in at_ready:
                            yield
                    nc.tensor.matmul(
                        pp[:],
                        AT_sb[:, hc, tt * P:(tt + 1) * P],
                        wp_sb[:, hc, ncol * 512:(ncol + 1) * 512],
                        start=(hc == 0), stop=(hc == NMC - 1),
                    )
                    if hc < NMC - 1:
                        yield
                ot = outsp.tile([P, 512], bf16, tag="ot", bufs=7)
                if tail:
                    nc.scalar.activation(ot[:], pp[:],
                                         mybir.ActivationFunctionType.Copy,
                                         scale=OUT_SCALE)
                else:
                    nc.vector.tensor_scalar(ot[:], pp[:], OUT_SCALE, None,
                                            mybir.AluOpType.mult)
                (deng or nc.sync).dma_start(
                    out_d[tt * P:(tt + 1) * P,
                          ncol * 512:(ncol + 1) * 512],
                    ot[:],
                )

            fillers = deque()   # [deadline, generator, started]

            def pump(n):
                while n > 0 and fillers:
                    ent = fillers[0]
                    try:
                        next(ent[1])
                        ent[2] = True
                    except StopIteration:
                        fillers.popleft()
                    n -= 1

            def pump_safe(n):
                """Advance only already-started chains (their PSUM slot is
                held; further matmuls have no new waits) -- safe to emit
                ahead of the scores matmuls without risking a new stall."""
                while n > 0 and fillers and fillers[0][2]:
                    try:
                        next(fillers[0][1])
                    except StopIteration:
                        fillers.popleft()
                    n -= 1

            def drain_until(deadline):
                while fillers and fillers[0][0] <= deadline:
                    for _ in fillers.popleft()[1]:
                        pass

            def run_gen(g):
                for _ in g:
                    pass

            # ---------------- startup compute ----------------
            for tt in range(4):
                run_gen(gen_v(tt))
            run_gen(gen_kq(False, 0, 0))
            run_gen(gen_kq(True, 0, 0))

            # deadline-ordered filler queue. Deadline = one group BEFORE
            # the (mch, qc) at whose start the output is first consumed,
            # so a slow pump can't leave the consuming group's first
            # scores matmul waiting on a just-drained projection chain.
            def prev_group(m, qc):
                # shift deadlines one group early so the consuming group's
                # first scores never wait on a just-drained chain -- but
                # not for m==0 (startup DMA still in flight there)
                if m == 0:
                    return (m, qc)
                return (m, qc - 1) if qc > 0 else (m - 1, NQC - 1)

            for m in range(NMC):
                for qc in range(NQC):
                    if m == 0 and qc == 0:
                        continue
                    dl = prev_group(m, qc)
                    fillers.append([dl, gen_kq(False, m, qc), False])
                    fillers.append([dl, gen_kq(True, m, qc), False])
                    if m == 0:
                        for tt in range(4 * qc, 4 * qc + 4):
                            fillers.append([dl, gen_v(tt), False])

            # ---------------- attention main loop ----------------
            def norm_qsub(pot, qsub):
                """Normalize accumulators (j=0,1) for q-subblock qsub into
                A natural layout; return the A_nat tile for transposition.
                pot slot order: 2*(qsub%2) + j."""
                s0 = 2 * (qsub % 2)
                rs = normp.tile([P, 2], f32, tag="rs")
                nc.vector.reciprocal(
                    rs[:], pot[:, s0:s0 + 2, HD:HD + 1])
                an = normp.tile([P, 2, HD], bf16, tag="an")
                rs_ap = rs[:]
                rs_b = bass.AP(
                    tensor=rs_ap.tensor, offset=rs_ap.offset,
                    ap=[list(rs_ap.ap[0]), [1, 2], [0, HD]])
                nc.vector.tensor_tensor(
                    an[:], pot[:, s0:s0 + 2, 0:HD], rs_b,
                    mybir.AluOpType.mult)
                return an

            def transpose_qsub(an, mch, qc, qsub):
                tr = ps_st.tile([P, P], bf16, tag="st", name="tr")
                nc.tensor.transpose(tr[:], an[:], eye_sb[:])
                nc.vector.tensor_copy(
                    AT_sb[:, mch, qc * 512 + qsub * P:
                          qc * 512 + (qsub + 1) * P], tr[:])
                if mch == NMC - 1:
                    at_ready.add(4 * qc + qsub)

            for mch in range(NMC):
                for qc in range(NQC):
                    drain_until((mch, qc))
                    # separate 1-bank tiles (separate pools): the next
                    # group's qsub0/1 tile only WARs against qsub0/1
                    # normalize reads, which happen early in this group
                    poA = ps_po.tile([P, 4, P], f32, tag="poA",
                                     name=f"poA{mch}_{qc}")
                    poB = ps_pb.tile([P, 4, P], f32, tag="poB",
                                     name=f"poB{mch}_{qc}")
                    po2 = [poA, poB]

                    def zero_po(half):
                        # one start=True matmul per 2KB accumulator bank
                        # (start zeroes a whole bank), emitted just before
                        # the bank's first P@V
                        nc.tensor.matmul(
                            po2[half][:, :, :],
                            zz[0:1, :, 0:P], zz[0:1, :, P:P + 512],
                            start=True, stop=True,
                            skip_group_check=True, perf_mode=DR,
                        )
                    if (mch, qc) == (NMC - 1, NQC - 1):
                        for s_ in range(4):
                            for ncol in range(2):
                                fillers.append(
                                    [(8, qc, s_),
                                     gen_out(12 + s_, ncol,
                                             tail=(ncol == 1),
                                             deng=(nc.scalar if ncol
                                                   else nc.sync),
                                             eager=True), False])
                    nki = 4 * qc + 4
                    pv_pending = deque()
                    pending_tr = []
                    deferred = []
                    cur_ki = [0]
                    zeroed = [False, False]

                    def pop_pv():
                        ki_, pts_ = pv_pending.popleft()
                        for half in range(2):
                            if not zeroed[half]:
                                zero_po(half)
                                zeroed[half] = True
                        do_pv(ki_, pts_)

                    def do_pv(ki, pts):
                        # P@V for k-block ki into accumulators (two-late)
                        for qsub in range(4):
                            qlim = 4 * qc + qsub
                            if ki > qlim:
                                continue
                            for j in range(2):
                                nc.tensor.matmul(
                                    po2[qsub // 2][:, 2 * (qsub % 2) + j,
                                                   0:HD + 1],
                                    pts[:, j, qsub * P:(qsub + 1) * P],
                                    V_sb[:, ki, 2 * mch + j, :],
                                    start=False, stop=(ki == qlim),
                                    skip_group_check=True,
                                )
                        if ki >= 4 * qc:
                            qsub = ki - 4 * qc
                            an = norm_qsub(po2[qsub // 2], qsub)
                            pending_tr.append((cur_ki[0], an, mch, qc, qsub))

                    for ki in range(nki):
                        off = max(0, ki - 4 * qc) * P
                        while deferred and deferred[0][0] <= ki:
                            fillers.append(deferred.pop(0)[1])
                        # continue in-flight filler chains ahead of the
                        # scores matmuls (no new waits -> absorbs ACT lag
                        # without risking a fresh-chain stall)
                        pump_safe(PUMP_SAFE_N)
                        pshat = ps_st.tile([P, 2, 512], f32, tag="st")
                        pts = ptp.tile([P, 2, 512], bf16, tag="pt")
                        for j in range(2):
                            part = j * 64
                            nc.tensor.matmul(
                                pshat[:, j, off:512],
                                KT_sb[part:part + 64, mch,
                                      ki * P:(ki + 1) * P],
                                QT_sb[part:part + 64, mch,
                                      qc * 512 + off:(qc + 1) * 512],
                                start=True, stop=True,
                            )
                        nc.scalar.activation(
                            pts[:, :, off:512], pshat[:, :, off:512],
                            mybir.ActivationFunctionType.Exp,
                            scale=EXP_SCALE, bias=ebias[:],
                        )
                        if ki >= 4 * qc:
                            # diagonal block: zero out q < k entries.
                            # On GpSimd (SBUF-only engine, otherwise idle)
                            # so the exp->mask chain never backs up the
                            # DVE queue; tri broadcast over j via a
                            # 0-stride AP dim.
                            tri_ap = tri_sb[:]
                            tri_b = bass.AP(
                                tensor=tri_ap.tensor, offset=tri_ap.offset,
                                ap=[list(tri_ap.ap[0]), [0, 2], [1, P]])
                            nc.vector.tensor_tensor(
                                pts[:, :, off:off + P],
                                pts[:, :, off:off + P],
                                tri_b, mybir.AluOpType.mult,
                            )
                        cur_ki[0] = ki
                        pump(8 if (mch, qc) == (NMC - 1, NQC - 1) else (PUMP_HI if mch == NMC - 1 else PUMP_LO))
                        # flush transposes whose normalize is >= 2 steps old
                        while pending_tr and pending_tr[0][0] <= ki - (1 if (mch, qc) == (NMC - 1, NQC - 1) else 3):
                            _, an_, m_, q_, s_ = pending_tr.pop(0)
                            transpose_qsub(an_, m_, q_, s_)
                            if mch == NMC - 1 and qc < NQC - 1:
                                # mch 3: this row-block's A^T is complete;
                                # its out-proj becomes pump fodder after a
                                # 2-iteration grace for the DVE copy
                                for ncol in range(2):
                                    deferred.append(
                                        (cur_ki[0] + 2,
                                         [(8, q_, s_),
                                          gen_out(4 * q_ + s_, ncol),
                                          False]))
                        pv_pending.append((ki, pts))
                        if len(pv_pending) > (1 if (mch, qc) == (NMC - 1, NQC - 1) else PV_LATE):
                            pop_pv()
                    # remaining k-blocks (two-late, post loop)
                    while pv_pending:
                        pop_pv()
                    while deferred:
                        fillers.append(deferred.pop(0)[1])
                    while pending_tr:
                        _, an_, m_, q_, s_ = pending_tr.pop(0)
                        transpose_qsub(an_, m_, q_, s_)
                        if mch == NMC - 1 and qc < NQC - 1:
                            for ncol in range(2):
                                fillers.append(
                                    [(8, q_, s_),
                                     gen_out(4 * q_ + s_, ncol), False])
            # ---- tail: drain remaining fillers round-robin ----
            wave = deque(e[1] for e in fillers)
            while wave:
                g = wave.popleft()
                try:
                    next(g)
                except StopIteration:
                    continue
                wave.append(g)

    nc.compile()
    return nc


def _split8(a):
    """fp32 -> (hi, lo) e4m3 split."""
    hi = a.astype(E4)
    lo = (a - hi.astype(np.float32)).astype(E4)
    return hi, lo


def host_inputs(x, w_qkv, b_qkv):
    """Per-core input maps. Core c -> batch c//2, head group c%2."""
    x = np.asarray(x, np.float32)
    w_qkv = np.asarray(w_qkv, np.float32) * WS
    b_qkv = np.asarray(b_qkv, np.float32) * WS
    tri = (np.arange(P)[None, :] >= np.arange(P)[:, None]).astype(BF16)
    eye = np.eye(P, dtype=np.float32).astype(BF16)

    # x^T regrouped to DoubleRow pairs: [p, c, i, t] = x[t, 256c+128i+p]
    xp_by_batch = []
    for b in range(B):
        xT = np.ascontiguousarray(x[b].T)          # [D, T]
        xr = xT.reshape(NDC2, 2, P, T).transpose(2, 0, 1, 3)  # [p,c,i,t]
        xp_by_batch.append(_split8(np.ascontiguousarray(xr)))

    wkq_by_g = []
    wv_by_g = []
    bqk_by_g = []
    bv_by_g = []
    for g in range(2):
        # interleaved [K_m | Q_m] 128-col pairs
        wkq = np.empty((D, 2 * GCOLS), np.float32)
        bqk = np.empty((P, 2 * NMC), np.float32)
        for m in range(NMC):
            qs = g * GCOLS + m * P
            ks = D + g * GCOLS + m * P
            wkq[:, 256 * m:256 * m + P] = w_qkv[:, ks:ks + P]
            wkq[:, 256 * m + P:256 * (m + 1)] = w_qkv[:, qs:qs + P]
            bqk[:, 2 * m] = b_qkv[ks:ks + P]
            bqk[:, 2 * m + 1] = b_qkv[qs:qs + P]
        # [d, col] -> [p, m, c, i, 256]: d = 256c+128i+p, col = 256m+w
        wkqr = wkq.reshape(NDC2, 2, P, NMC, 256).transpose(2, 3, 0, 1, 4)
        wkq_by_g.append(_split8(np.ascontiguousarray(wkqr)))
        wv = np.ascontiguousarray(
            w_qkv[:, 2 * D + g * GCOLS: 2 * D + (g + 1) * GCOLS])
        wvr = wv.reshape(NDC2, 2, P, GCOLS).transpose(2, 0, 1, 3)
        wv_by_g.append(_split8(np.ascontiguousarray(wvr)))
        bqk_by_g.append(bqk)
        bv_by_g.append(np.ascontiguousarray(
            b_qkv[2 * D + g * GCOLS: 2 * D + (g + 1) * GCOLS]).astype(
                np.float32))

    in_maps = []
    for c in range(NCORES):
        b, g = c // 2, c % 2
        xh, xl = xp_by_batch[b]
        wkqh, wkql = wkq_by_g[g]
        wvh, wvl = wv_by_g[g]
        in_maps.append({
            "xh": xh, "xl": xl,
            "wkqh": wkqh, "wkql": wkql,
            "wvh": wvh, "wvl": wvl,
            "wp": None,  # filled by caller (needs w_proj)
            "bqk": bqk_by_g[g], "bv": bv_by_g[g],
            "tri": tri, "eye": eye,
        })
    return in_maps


def full_in_maps(x, w_qkv, b_qkv, w_proj):
    w_proj = np.asarray(w_proj, np.float32) * WS
    in_maps = host_inputs(x, w_qkv, b_qkv)
    for c in range(NCORES):
        g = c % 2
        in_maps[c]["wp"] = np.ascontiguousarray(
            w_proj[g * GCOLS:(g + 1) * GCOLS, :]).astype(BF16)
    return in_maps


def gather(results, b_proj):
    out = np.zeros((B, T, D), np.float32)
    for c in range(NCORES):
        out[c // 2] += results[c]["outp"].astype(np.float32)
    out += np.asarray(b_proj, np.float32)[None, None, :]
    return out


_NC_CACHE = None


def kernel(x, w_qkv, b_qkv, w_proj, b_proj):
    global _NC_CACHE
    if _NC_CACHE is None:
        _NC_CACHE = build_nc()
    in_maps = full_in_maps(x, w_qkv, b_qkv, w_proj)
    res = run_bass_kernel_spmd(_NC_CACHE, in_maps, core_ids=list(range(NCORES)))
    return gather(res.results, b_proj)


if __name__ == "__main__":
    rng = np.random.default_rng(0)
    x = rng.standard_normal((B, T, D), dtype=np.float32)
    w_qkv = rng.standard_normal((D, 3 * D), dtype=np.float32) / np.sqrt(D)
    b_qkv = np.zeros(3 * D, np.float32)
    w_proj = rng.standard_normal((D, D), dtype=np.float32) / np.sqrt(D)
    b_proj = np.zeros(D, np.float32)
    out = kernel(x, w_qkv, b_qkv, w_proj, b_proj)
    print(out.shape, out.dtype)
